# revision 8
# baseline (speedup 1.0000x reference)
"""ArcDecoder Bass kernel for 8 TRN2 NeuronCores.

Math (norm_w folded into weights host-side; norm_b==0 for this problem,
general case falls back to numpy):
  zn = LN(z); u1 = relu(zn@W1eff+b1); u2 = relu(zn@W2eff+b2)
  h1n = LN(u1); h2n = LN(u2); g = h1n @ Wbil_eff
  score_e = dot(g[a0_e], h2n[a1_e]) + bil_b

Phase A (node tables, sharded): core c computes nodes [c*6272,(c+1)*6272)
  from its own z-shard.  LN stats via batched square+reduce (7-tile
  chunks), normalization applied on the ACT engine (scale/bias APs).
  g-shard -> DRAM -> AllGather into a device-wide table; the h2 applies
  are deferred until after the AllGather is issued so they overlap it.
Phase B (edges assigned to the core owning their TAIL shard):
  * heads: SWDGE dma_gather, 1024 rows/block round-robin over 4 queues;
    all int16 indices preloaded in one DMA.
  * tails: per 1024-edge block one K=1 broadcast matmul pair + one
    1024-wide is_equal builds the one-hot sT; 8 back-to-back selection
    matmuls land in a single [128,1024] PSUM tile.
  * one DVE multiply + one reduce per block -> scores accumulate in
    SBUF; a single DMA stores all scores at the end.  Host adds bil_b
    and inverse-permutes back to input edge order.
"""

import sys

if "/opt/trn_rl_repo" not in sys.path:
    sys.path.insert(0, "/opt/trn_rl_repo")

import numpy as np
import ml_dtypes

import concourse.bass as bass
import concourse.tile as tile
from concourse import bacc, mybir
from concourse.bass_utils import run_bass_kernel_spmd
from concourse.library_config import mlp
from concourse.masks import make_identity

N, D, E = 50000, 128, 500000
NCORES = 8
P = 128
SHARD_T = 49                     # node tiles per core (sharded phase A)
SHARD = SHARD_T * P              # 6272 rows per core
NT = SHARD_T * NCORES            # 392 node tiles total
NPAD = NT * P                    # 50176
CH = 7                           # tiles per phase-A chunk
EC = E // NCORES                 # 62500 edges per core
HALF = 32768                     # int16 index ceiling for dma_gather

F32 = mybir.dt.float32
BF16 = mybir.dt.bfloat16
AF = mybir.ActivationFunctionType
ALU = mybir.AluOpType

TABLE_DT = BF16                  # dtype of g table (gather payload)
MM_DT = BF16                     # dtype of matmul operands
BLK = 1024                       # edges per gather block
CB = BLK // P                    # tiles per gather block
NQ = 4                           # SWDGE queues
LIDX_CHUNK = 4                   # blocks per lidx row load
EPS = 1e-5

_np_tdt = np.float32 if TABLE_DT == F32 else ml_dtypes.bfloat16
_np_mdt = np.float32 if MM_DT == F32 else ml_dtypes.bfloat16


def _build(block_specs):
    """block_specs: list of (head_half, chunk-tuple) per 1024-edge block."""
    nblk = len(block_specs)
    ecpad = nblk * BLK
    S = BLK // 16
    nlc = -(-nblk // LIDX_CHUNK)

    nc = bacc.Bacc("TRN2", target_bir_lowering=False, debug=False,
                   num_devices=NCORES, num_swdge_queues=NQ)

    z_ext = nc.dram_tensor("z", [P, SHARD_T * D], F32, kind="ExternalInput").ap()
    wcat_ext = nc.dram_tensor("wcat", [D, 2 * D], MM_DT, kind="ExternalInput").ap()
    bcat_ext = nc.dram_tensor("bcat", [1, 2 * D], MM_DT, kind="ExternalInput").ap()
    wbil_ext = nc.dram_tensor("wbil", [D, D], MM_DT, kind="ExternalInput").ap()
    iotac_ext = nc.dram_tensor("iotac", [P, 1], F32, kind="ExternalInput").ap()
    idx0_ext = nc.dram_tensor("idx0", [P, nblk, S], mybir.dt.int16,
                              kind="ExternalInput").ap()
    lidx_ext = nc.dram_tensor("lidx", [nblk, BLK], mybir.dt.int8,
                              kind="ExternalInput").ap()
    iota8_ext = nc.dram_tensor("iota8", [P, 1], mybir.dt.int8,
                               kind="ExternalInput").ap()
    out_ext = nc.dram_tensor("out", [P, ecpad // P], F32, kind="ExternalOutput").ap()

    g_shard = nc.dram_tensor("g_shard", [SHARD, D], TABLE_DT).ap()
    g_table = nc.dram_tensor("g_table", [NPAD, D], TABLE_DT, addr_space="Shared").ap()

    with tile.TileContext(nc) as tc:
        with (
            tc.tile_pool(name="const", bufs=1) as const_p,
            tc.tile_pool(name="zload", bufs=2) as zload_p,
            tc.tile_pool(name="stat", bufs=2) as stat_p,
            tc.tile_pool(name="work", bufs=8) as work_p,
            tc.tile_pool(name="gstore", bufs=2) as gstore_p,
            tc.tile_pool(name="gather", bufs=12) as gather_p,
            tc.tile_pool(name="mask", bufs=4) as mask_p,
            tc.tile_pool(name="lrow", bufs=4) as lrow_p,
            tc.tile_pool(name="prod", bufs=4) as prod_p,
        ):
            # ---- constants / preloads ---------------------------------
            nc.gpsimd.load_library(mlp)
            ident = const_p.tile([P, P], MM_DT)
            make_identity(nc, ident[:])
            wcat_sb = const_p.tile([D, 2 * D], MM_DT)
            nc.sync.dma_start(wcat_sb[:], wcat_ext[:])
            bcat_sb = const_p.tile([1, 2 * D], MM_DT)
            nc.sync.dma_start(bcat_sb[:], bcat_ext[:])
            wbil_sb = const_p.tile([D, D], MM_DT)
            nc.sync.dma_start(wbil_sb[:], wbil_ext[:])
            ones_row = const_p.tile([1, P], MM_DT)
            nc.vector.memset(ones_row[:], 1.0)
            epsc = const_p.tile([P, 1], F32)
            nc.vector.memset(epsc[:], EPS)
            iotac = const_p.tile([P, 1], F32)
            nc.sync.dma_start(iotac[:], iotac_ext[:])
            iota8 = const_p.tile([P, 1], mybir.dt.int8)
            nc.sync.dma_start(iota8[:], iota8_ext[:])
            idx0_sb = const_p.tile([P, nblk, S], mybir.dt.int16)
            nc.sync.dma_start(idx0_sb[:], idx0_ext[:])

            h_sb = const_p.tile([P, SHARD_T, D], MM_DT)
            u12buf = const_p.tile([P, SHARD_T, 2 * D], MM_DT)
            scorebuf = const_p.tile([P, nblk, CB], F32)
            # per-tile LN scalars (columns): scale r and bias -mu*r
            r1b = const_p.tile([P, SHARD_T], F32)
            n1b = const_p.tile([P, SHARD_T], F32)
            r2b = const_p.tile([P, SHARD_T], F32)
            n2b = const_p.tile([P, SHARD_T], F32)

            pa = tc.tile_pool(name="psumA", bufs=2, space="PSUM")
            psum_p = pa.__enter__()

            # ---- phase A: z -> u12 -> g (h2 deferred) ------------------
            for c0 in range(0, SHARD_T, CH):
                ct = min(CH, SHARD_T - c0)
                zb = zload_p.tile([P, CH, D], F32, tag="zb")
                nc.sync.dma_start(
                    zb[:, :ct, :],
                    z_ext[:, c0 * D:(c0 + ct) * D].rearrange(
                        "p (j d) -> p j d", d=D),
                )
                # LN1 stats: mean/var via square + reduce
                sq = stat_p.tile([P, CH, D], F32, tag="sq")
                nc.vector.tensor_tensor(sq[:, :ct, :], zb[:, :ct, :],
                                        zb[:, :ct, :], op=ALU.mult)
                sm = stat_p.tile([P, CH], F32, tag="sm")
                nc.vector.tensor_reduce(sm[:, :ct], zb[:, :ct, :],
                                        axis=mybir.AxisListType.X, op=ALU.add)
                sq2 = stat_p.tile([P, CH], F32, tag="sq2")
                nc.vector.tensor_reduce(sq2[:, :ct], sq[:, :ct, :],
                                        axis=mybir.AxisListType.X, op=ALU.add)
                mu = stat_p.tile([P, CH], F32, tag="mu")
                nc.vector.tensor_scalar(mu[:, :ct], sm[:, :ct], 1.0 / D, None,
                                        ALU.mult)
                var = stat_p.tile([P, CH], F32, tag="var")
                nc.vector.tensor_tensor(var[:, :ct], mu[:, :ct], mu[:, :ct],
                                        op=ALU.mult)
                nc.vector.tensor_scalar(sq2[:, :ct], sq2[:, :ct], 1.0 / D,
                                        None, ALU.mult)
                nc.vector.tensor_tensor(var[:, :ct], sq2[:, :ct], var[:, :ct],
                                        op=ALU.subtract)
                sd = stat_p.tile([P, CH], F32, tag="sd")
                nc.scalar.activation(sd[:, :ct], var[:, :ct], AF.Sqrt,
                                     bias=epsc[:])
                r0 = stat_p.tile([P, CH], F32, tag="r0")
                nc.vector.reciprocal(r0[:, :ct], sd[:, :ct])
                n0 = stat_p.tile([P, CH], F32, tag="n0")
                nc.vector.tensor_scalar(n0[:, :ct], mu[:, :ct], -1.0, None,
                                        ALU.mult)
                nc.vector.tensor_tensor(n0[:, :ct], n0[:, :ct], r0[:, :ct],
                                        op=ALU.mult)

                for jj in range(ct):
                    t = c0 + jj
                    zn = work_p.tile([P, D], MM_DT, tag="zn")
                    nc.scalar.activation(zn[:], zb[:, jj, :], AF.Identity,
                                         bias=n0[:, jj:jj + 1],
                                         scale=r0[:, jj:jj + 1])
                    znT_ps = psum_p.tile([P, P], MM_DT, tag="tpos", space="PSUM")
                    nc.tensor.transpose(znT_ps[:], zn[:], ident[:])
                    znT = work_p.tile([P, P], MM_DT, tag="znT")
                    nc.vector.tensor_copy(znT[:], znT_ps[:])
                    u12_ps = psum_p.tile([P, 2 * D], F32, tag="mm12", space="PSUM")
                    nc.tensor.matmul(u12_ps[:], lhsT=znT[:], rhs=wcat_sb[:],
                                     start=True, stop=False)
                    nc.tensor.matmul(u12_ps[:], lhsT=ones_row[:], rhs=bcat_sb[:],
                                     start=False, stop=True)
                    nc.scalar.activation(u12buf[:, t, :], u12_ps[:], AF.Relu)

                # LN2/LN3 stats on the chunk's u12 (bf16 in, fp32 reduce)
                u12c = u12buf[:, c0:c0 + ct, :].rearrange(
                    "p t (h d) -> p (t h) d", d=D)
                sqh = stat_p.tile([P, 2 * CH, D], MM_DT, tag="sqh")
                nc.vector.tensor_tensor(sqh[:, :2 * ct, :], u12c, u12c,
                                        op=ALU.mult)
                smh = stat_p.tile([P, 2 * CH], F32, tag="smh")
                nc.vector.tensor_reduce(smh[:, :2 * ct], u12c,
                                        axis=mybir.AxisListType.X, op=ALU.add)
                sqh2 = stat_p.tile([P, 2 * CH], F32, tag="sqh2")
                nc.vector.tensor_reduce(sqh2[:, :2 * ct], sqh[:, :2 * ct, :],
                                        axis=mybir.AxisListType.X, op=ALU.add)
                muh = stat_p.tile([P, 2 * CH], F32, tag="muh")
                nc.vector.tensor_scalar(muh[:, :2 * ct], smh[:, :2 * ct],
                                        1.0 / D, None, ALU.mult)
                varh = stat_p.tile([P, 2 * CH], F32, tag="varh")
                nc.vector.tensor_tensor(varh[:, :2 * ct], muh[:, :2 * ct],
                                        muh[:, :2 * ct], op=ALU.mult)
                nc.vector.tensor_scalar(sqh2[:, :2 * ct], sqh2[:, :2 * ct],
                                        1.0 / D, None, ALU.mult)
                nc.vector.tensor_tensor(varh[:, :2 * ct], sqh2[:, :2 * ct],
                                        varh[:, :2 * ct], op=ALU.subtract)
                sdh = stat_p.tile([P, 2 * CH], F32, tag="sdh")
                nc.scalar.activation(sdh[:, :2 * ct], varh[:, :2 * ct],
                                     AF.Sqrt, bias=epsc[:])
                rh = stat_p.tile([P, 2 * CH], F32, tag="rh")
                nc.vector.reciprocal(rh[:, :2 * ct], sdh[:, :2 * ct])
                nh = stat_p.tile([P, 2 * CH], F32, tag="nh")
                nc.vector.tensor_scalar(nh[:, :2 * ct], muh[:, :2 * ct], -1.0,
                                        None, ALU.mult)
                nc.vector.tensor_tensor(nh[:, :2 * ct], nh[:, :2 * ct],
                                        rh[:, :2 * ct], op=ALU.mult)
                # stash per-tile columns: (t h) layout -> h=0 is u1, h=1 u2
                nc.vector.tensor_copy(
                    r1b[:, c0:c0 + ct],
                    rh[:, 0:2 * ct].rearrange("p (t h) -> p t h", h=2)[:, :, 0])
                nc.vector.tensor_copy(
                    n1b[:, c0:c0 + ct],
                    nh[:, 0:2 * ct].rearrange("p (t h) -> p t h", h=2)[:, :, 0])
                nc.vector.tensor_copy(
                    r2b[:, c0:c0 + ct],
                    rh[:, 0:2 * ct].rearrange("p (t h) -> p t h", h=2)[:, :, 1])
                nc.vector.tensor_copy(
                    n2b[:, c0:c0 + ct],
                    nh[:, 0:2 * ct].rearrange("p (t h) -> p t h", h=2)[:, :, 1])

                # g path for the chunk
                gbuf = gstore_p.tile([P, CH, D], TABLE_DT, tag="gbuf")
                for jj in range(ct):
                    t = c0 + jj
                    h1n = work_p.tile([P, D], MM_DT, tag="h1n")
                    nc.scalar.activation(h1n[:], u12buf[:, t, 0:D], AF.Identity,
                                         bias=n1b[:, t:t + 1],
                                         scale=r1b[:, t:t + 1])
                    h1nT_ps = psum_p.tile([P, P], MM_DT, tag="tpos", space="PSUM")
                    nc.tensor.transpose(h1nT_ps[:], h1n[:], ident[:])
                    h1nT = work_p.tile([P, P], MM_DT, tag="h1nT")
                    nc.vector.tensor_copy(h1nT[:], h1nT_ps[:])
                    g_ps = psum_p.tile([P, D], F32, tag="gmm", space="PSUM")
                    nc.tensor.matmul(g_ps[:], lhsT=h1nT[:], rhs=wbil_sb[:],
                                     start=True, stop=True)
                    nc.vector.tensor_copy(gbuf[:, jj, :], g_ps[:])
                nc.sync.dma_start(
                    g_shard.rearrange("(p j) d -> p j d", p=P)[:, c0:c0 + ct, :],
                    gbuf[:, :ct, :],
                )

            # ---- all-gather g shards into the full table ---------------
            nc.gpsimd.collective_compute(
                "AllGather", ALU.bypass,
                replica_groups=[list(range(NCORES))],
                ins=[g_shard[:]], outs=[g_table[:]],
            )

            # ---- deferred h2 applies (overlap the collective) ----------
            for t in range(SHARD_T):
                nc.scalar.activation(h_sb[:, t, :], u12buf[:, t, D:2 * D],
                                     AF.Identity, bias=n2b[:, t:t + 1],
                                     scale=r2b[:, t:t + 1])

            pa.__exit__(None, None, None)
            pb = tc.tile_pool(name="psumB", bufs=2, space="PSUM")
            psum_p = pb.__enter__()

            # ---- phase B ----------------------------------------------
            for b, (h0, chunks) in enumerate(block_specs):
                lb8 = lrow_p.tile([P, BLK], mybir.dt.int8, tag="lb8")
                nc.scalar.dma_start(lb8[:],
                                    lidx_ext[b][None, :].to_broadcast((P, BLK)))
                g_src = g_table[HALF:, :] if h0 else g_table[:, :]
                gt = gather_p.tile([P, CB, D], TABLE_DT, tag="gt")
                nc.gpsimd.dma_gather(gt[:], g_src, idx0_sb[:, b, :], BLK, BLK,
                                     D, queue_num=b % NQ)
                sT = mask_p.tile([P, BLK], MM_DT, tag="sT")
                nc.vector.tensor_scalar(sT[:], lb8[:], iotac[:], None,
                                        ALU.is_equal)
                hsel_ps = psum_p.tile([P, BLK], F32, tag="hsel", space="PSUM")
                for j, ch in enumerate(chunks):
                    nc.tensor.matmul(hsel_ps[:, j * P:(j + 1) * P],
                                     lhsT=sT[:, j * P:(j + 1) * P],
                                     rhs=h_sb[:, ch, :], start=True, stop=True)
                prod = prod_p.tile([P, CB, D], MM_DT, tag="prod")
                nc.vector.tensor_tensor(
                    prod[:].rearrange("p c d -> p (c d)"),
                    gt[:].rearrange("p c d -> p (c d)"),
                    hsel_ps[:], op=ALU.mult)
                nc.vector.tensor_reduce(
                    scorebuf[:, b, :], prod[:], axis=mybir.AxisListType.X,
                    op=ALU.add)

            nc.sync.dma_start(
                out_ext[:],
                scorebuf[:].rearrange("p b c -> p (b c)"),
            )

            pb.__exit__(None, None, None)

    nc.compile()
    return nc


_CACHE = {}
_RUN_KWARGS = {}
LAST_RESULTS = None


def _pack_idx(vals):
    """[nblk, 1024] int16 -> dma_gather SBUF layout [nblk, 128, 64]:
    index k lives at partition k%16, column k//16, replicated into all
    eight 16-partition groups."""
    nblk = vals.shape[0]
    w = vals.reshape(nblk, BLK // 16, 16).transpose(0, 2, 1)   # [nblk,16,S]
    return np.tile(w, (1, 8, 1)).astype(np.int16)


def kernel(**inputs) -> np.ndarray:
    z = np.asarray(inputs["z"], np.float32)
    pot_arcs = np.asarray(inputs["pot_arcs"])
    lin1_w = np.asarray(inputs["lin1_w"], np.float32)
    lin1_b = np.asarray(inputs["lin1_b"], np.float32)
    lin2_w = np.asarray(inputs["lin2_w"], np.float32)
    lin2_b = np.asarray(inputs["lin2_b"], np.float32)
    bil_w = np.asarray(inputs["bil_w"], np.float32)
    bil_b = np.asarray(inputs["bil_b"], np.float32)
    norm_w = np.asarray(inputs["norm_w"], np.float32)
    norm_b = np.asarray(inputs["norm_b"], np.float32)

    if not np.allclose(norm_b, 0.0):
        # general norm_b adds per-node scalar terms; not exercised by this
        # problem's inputs.  Exact numpy fallback keeps kernel() total.
        return _numpy_reference(z, pot_arcs, lin1_w, lin1_b, lin2_w, lin2_b,
                                bil_w, bil_b, norm_w, norm_b)

    w1eff = norm_w[:, None] * lin1_w.T
    b1eff = norm_b @ lin1_w.T + lin1_b
    w2eff = norm_w[:, None] * lin2_w.T
    b2eff = norm_b @ lin2_w.T + lin2_b
    wbil = bil_w[0] * norm_w[None, :]
    wcat = np.concatenate([w1eff, w2eff], axis=1).astype(_np_mdt)
    bcat = np.concatenate([b1eff, b2eff])[None, :].astype(_np_mdt)
    wbil = wbil.astype(_np_mdt)

    zpad = np.zeros((NPAD, D), np.float32)
    zpad[:N] = z
    # row remap: node n -> table row owner*SHARD + (n%SHARD)%P*SHARD_T + (n%SHARD)//P
    noff = np.arange(NPAD, dtype=np.int32) % SHARD
    rowmap = (np.arange(NPAD, dtype=np.int32) // SHARD) * SHARD + \
        (noff % P) * SHARD_T + noff // P

    a0 = pot_arcs[:, 0].astype(np.int32)
    a1 = pot_arcs[:, 1].astype(np.int32)

    # --- assign edges to tail-owner cores; tile by (head-half, tail-chunk) -
    core_of_edge = a1 // SHARD
    per_core = []
    for c in range(NCORES):
        eids = np.where(core_of_edge == c)[0]
        a0c = rowmap[a0[eids]]
        l1 = a1[eids] - c * SHARD
        bucket = (a0c >= HALF).astype(np.int64)
        chunk = l1 // P
        # sort by (bucket, chunk) and cut chunk-pure 128-edge tiles
        order = np.lexsort((chunk, bucket))
        per_core.append((eids, a0c, l1, bucket, chunk, order))

    # tiles per (bucket, chunk) must be uniform across cores (same program)
    ntile_bc = np.zeros((2, SHARD_T), np.int64)
    for c in range(NCORES):
        _, _, _, bucket, chunk, _ = per_core[c]
        for bkt in range(2):
            cnt = np.bincount(chunk[bucket == bkt], minlength=SHARD_T)
            ntile_bc[bkt] = np.maximum(ntile_bc[bkt], -(-cnt // P))
    # tile list: (bucket, chunk) repeated; pad each bucket to multiple of 8
    tiles = []
    for bkt in range(2):
        start = len(tiles)
        for ch in range(SHARD_T):
            tiles += [(bkt, ch)] * int(ntile_bc[bkt, ch])
        while (len(tiles) - start) % CB:
            tiles.append((bkt, 0))
    ntiles = len(tiles)
    nblk = ntiles // CB
    ecpad = ntiles * P

    block_specs = []
    for b in range(nblk):
        bts = tiles[b * CB:(b + 1) * CB]
        assert len({t[0] for t in bts}) == 1
        block_specs.append((bts[0][0], tuple(t[1] for t in bts)))

    # slot ranges per (bucket, chunk)
    tile_start = {}
    pos = 0
    for t in tiles:
        tile_start.setdefault(t, []).append(pos)
        pos += P

    in_maps = []
    gathers = []
    iota_col = np.arange(P, dtype=np.float32).reshape(P, 1)
    for c in range(NCORES):
        eids, a0c, l1, bucket, chunk, order = per_core[c]
        i0 = np.zeros(ecpad, np.int32)
        lidx = np.zeros(ecpad, np.int32)
        gid = np.empty(len(eids), np.int64)
        for bkt in range(2):
            for ch in range(SHARD_T):
                sel = order[(bucket[order] == bkt) & (chunk[order] == ch)]
                starts = tile_start[(bkt, ch)]
                for ti in range(len(starts)):
                    seg = sel[ti * P:(ti + 1) * P]
                    dst = starts[ti] + np.arange(len(seg))
                    i0[dst] = a0c[seg] - (HALF if bkt else 0)
                    lidx[dst] = l1[seg] - ch * P
                    gid[seg] = dst
        gathers.append((eids, gid))
        in_maps.append({
            "z": zpad[c * SHARD:(c + 1) * SHARD].reshape(
                SHARD_T, P, D).transpose(1, 0, 2).reshape(P, SHARD_T * D).copy(),
            "wcat": wcat,
            "bcat": bcat,
            "wbil": wbil,
            "iotac": iota_col,
            "idx0": _pack_idx(i0.astype(np.int16).reshape(nblk, BLK)).transpose(1, 0, 2).copy(),
            "lidx": lidx.astype(np.int8).reshape(nblk, BLK),
            "iota8": np.arange(P, dtype=np.int8).reshape(P, 1),
        })

    key = tuple(block_specs)
    if key not in _CACHE:
        _CACHE[key] = _build(block_specs)
    nc = _CACHE[key]

    res = run_bass_kernel_spmd(nc, in_maps, list(range(NCORES)), **_RUN_KWARGS)
    global LAST_RESULTS
    LAST_RESULTS = res

    scores = np.empty(E, np.float32)
    for c in range(NCORES):
        out_c = np.asarray(res.results[c]["out"], np.float32).T.ravel()
        eids, gid = gathers[c]
        scores[eids] = out_c[gid]
    return scores + float(bil_b[0])


def _numpy_reference(z, pot_arcs, lin1_w, lin1_b, lin2_w, lin2_b,
                     bil_w, bil_b, norm_w, norm_b):
    def ln(x):
        mu = x.mean(-1, keepdims=True)
        var = x.var(-1, keepdims=True)
        return (x - mu) / np.sqrt(var + 1e-5) * norm_w + norm_b

    zn = ln(z)
    h1 = ln(np.maximum(zn @ lin1_w.T + lin1_b, 0.0))
    h2 = ln(np.maximum(zn @ lin2_w.T + lin2_b, 0.0))
    g = h1 @ bil_w[0]
    a0 = pot_arcs[:, 0].astype(np.int64)
    a1 = pot_arcs[:, 1].astype(np.int64)
    return np.einsum("ed,ed->e", g[a0], h2[a1]) + bil_b[0]


# revision 9
# speedup vs baseline: 1.1715x; 1.1715x over previous
"""ArcDecoder Bass kernel for 8 TRN2 NeuronCores.

Math (norm_w folded into weights host-side; norm_b==0 for this problem,
general case falls back to numpy):
  zn = LN(z); u1 = relu(zn@W1eff+b1); u2 = relu(zn@W2eff+b2)
  h1n = LN(u1); h2n = LN(u2); g = h1n @ Wbil_eff
  score_e = dot(g[a0_e], h2n[a1_e]) + bil_b

Phase A (node tables, sharded): core c computes nodes [c*6272,(c+1)*6272)
  from its own z-shard.  LN stats via batched square+reduce (7-tile
  chunks), normalization applied on the ACT engine (scale/bias APs).
  g-shard -> DRAM -> AllGather into a device-wide table; the h2 applies
  are deferred until after the AllGather is issued so they overlap it.
Phase B (edges assigned to the core owning their TAIL shard):
  * heads: SWDGE dma_gather, 1024 rows/block round-robin over 4 queues;
    all int16 indices preloaded in one DMA.
  * tails: per 1024-edge block one K=1 broadcast matmul pair + one
    1024-wide is_equal builds the one-hot sT; 8 back-to-back selection
    matmuls land in a single [128,1024] PSUM tile.
  * one DVE multiply + one reduce per block -> scores accumulate in
    SBUF; a single DMA stores all scores at the end.  Host adds bil_b
    and inverse-permutes back to input edge order.
"""

import sys

if "/opt/trn_rl_repo" not in sys.path:
    sys.path.insert(0, "/opt/trn_rl_repo")

import numpy as np
import ml_dtypes

import concourse.bass as bass
import concourse.tile as tile
from concourse import bacc, mybir
from concourse.bass_utils import run_bass_kernel_spmd
from concourse.library_config import mlp
from concourse.masks import make_identity

N, D, E = 50000, 128, 500000
NCORES = 8
P = 128
SHARD_T = 49                     # node tiles per core (sharded phase A)
SHARD = SHARD_T * P              # 6272 rows per core
NT = SHARD_T * NCORES            # 392 node tiles total
NPAD = NT * P                    # 50176
CH = 7                           # tiles per phase-A chunk
EC = E // NCORES                 # 62500 edges per core
HALF = 32768                     # int16 index ceiling for dma_gather

F32 = mybir.dt.float32
BF16 = mybir.dt.bfloat16
AF = mybir.ActivationFunctionType
ALU = mybir.AluOpType

TABLE_DT = BF16                  # dtype of g table (gather payload)
MM_DT = BF16                     # dtype of matmul operands
BLK = 1024                       # edges per gather block
CB = BLK // P                    # tiles per gather block
NQ = 4                           # SWDGE queues
LIDX_CHUNK = 4                   # blocks per lidx row load
EPS = 1e-5

_np_tdt = np.float32 if TABLE_DT == F32 else ml_dtypes.bfloat16
_np_mdt = np.float32 if MM_DT == F32 else ml_dtypes.bfloat16


def _build(block_specs):
    """block_specs: list of (head_half, chunk-tuple) per 1024-edge block."""
    nblk = len(block_specs)
    ecpad = nblk * BLK
    S = BLK // 16
    nlc = -(-nblk // LIDX_CHUNK)

    nc = bacc.Bacc("TRN2", target_bir_lowering=False, debug=False,
                   num_devices=NCORES, num_swdge_queues=NQ)

    z_ext = nc.dram_tensor("z", [P, SHARD_T * D], F32, kind="ExternalInput").ap()
    wcat_ext = nc.dram_tensor("wcat", [D, 2 * D], MM_DT, kind="ExternalInput").ap()
    bcat_ext = nc.dram_tensor("bcat", [1, 2 * D], MM_DT, kind="ExternalInput").ap()
    wbil_ext = nc.dram_tensor("wbil", [D, D], MM_DT, kind="ExternalInput").ap()
    iotac_ext = nc.dram_tensor("iotac", [P, 1], F32, kind="ExternalInput").ap()
    idx0_ext = nc.dram_tensor("idx0", [P, nblk, S], mybir.dt.int16,
                              kind="ExternalInput").ap()
    lidx_ext = nc.dram_tensor("lidx", [nlc, LIDX_CHUNK * BLK], MM_DT,
                              kind="ExternalInput").ap()
    out_ext = nc.dram_tensor("out", [P, ecpad // P], F32, kind="ExternalOutput").ap()

    g_shard = nc.dram_tensor("g_shard", [SHARD, D], TABLE_DT).ap()
    g_table = nc.dram_tensor("g_table", [NPAD, D], TABLE_DT, addr_space="Shared").ap()

    with tile.TileContext(nc) as tc:
        with (
            tc.tile_pool(name="const", bufs=1) as const_p,
            tc.tile_pool(name="zload", bufs=2) as zload_p,
            tc.tile_pool(name="stat", bufs=2) as stat_p,
            tc.tile_pool(name="work", bufs=8) as work_p,
            tc.tile_pool(name="gstore", bufs=2) as gstore_p,
            tc.tile_pool(name="gather", bufs=12) as gather_p,
            tc.tile_pool(name="mask", bufs=4) as mask_p,
            tc.tile_pool(name="lrow", bufs=2) as lrow_p,
            tc.tile_pool(name="prod", bufs=4) as prod_p,
        ):
            # ---- constants / preloads ---------------------------------
            nc.gpsimd.load_library(mlp)
            ident = const_p.tile([P, P], MM_DT)
            make_identity(nc, ident[:])
            wcat_sb = const_p.tile([D, 2 * D], MM_DT)
            nc.sync.dma_start(wcat_sb[:], wcat_ext[:])
            bcat_sb = const_p.tile([1, 2 * D], MM_DT)
            nc.sync.dma_start(bcat_sb[:], bcat_ext[:])
            wbil_sb = const_p.tile([D, D], MM_DT)
            nc.sync.dma_start(wbil_sb[:], wbil_ext[:])
            ones_row = const_p.tile([1, P], MM_DT)
            nc.vector.memset(ones_row[:], 1.0)
            epsc = const_p.tile([P, 1], F32)
            nc.vector.memset(epsc[:], EPS)
            iotac = const_p.tile([P, 1], F32)
            nc.sync.dma_start(iotac[:], iotac_ext[:])
            idx0_sb = const_p.tile([P, nblk, S], mybir.dt.int16)
            nc.sync.dma_start(idx0_sb[:], idx0_ext[:])

            h_sb = const_p.tile([P, SHARD_T, D], MM_DT)
            u12buf = const_p.tile([P, SHARD_T, 2 * D], MM_DT)
            scorebuf = const_p.tile([P, nblk, CB], F32)
            # per-tile LN scalars (columns): scale r and bias -mu*r
            r1b = const_p.tile([P, SHARD_T], F32)
            n1b = const_p.tile([P, SHARD_T], F32)
            r2b = const_p.tile([P, SHARD_T], F32)
            n2b = const_p.tile([P, SHARD_T], F32)

            pa = tc.tile_pool(name="psumA", bufs=2, space="PSUM")
            psum_p = pa.__enter__()

            # ---- phase A: z -> u12 -> g (h2 deferred) ------------------
            for c0 in range(0, SHARD_T, CH):
                ct = min(CH, SHARD_T - c0)
                zb = zload_p.tile([P, CH, D], F32, tag="zb")
                nc.sync.dma_start(
                    zb[:, :ct, :],
                    z_ext[:, c0 * D:(c0 + ct) * D].rearrange(
                        "p (j d) -> p j d", d=D),
                )
                # LN1 stats: mean/var via square + reduce
                sq = stat_p.tile([P, CH, D], F32, tag="sq")
                nc.vector.tensor_tensor(sq[:, :ct, :], zb[:, :ct, :],
                                        zb[:, :ct, :], op=ALU.mult)
                sm = stat_p.tile([P, CH], F32, tag="sm")
                nc.vector.tensor_reduce(sm[:, :ct], zb[:, :ct, :],
                                        axis=mybir.AxisListType.X, op=ALU.add)
                sq2 = stat_p.tile([P, CH], F32, tag="sq2")
                nc.vector.tensor_reduce(sq2[:, :ct], sq[:, :ct, :],
                                        axis=mybir.AxisListType.X, op=ALU.add)
                mu = stat_p.tile([P, CH], F32, tag="mu")
                nc.vector.tensor_scalar(mu[:, :ct], sm[:, :ct], 1.0 / D, None,
                                        ALU.mult)
                var = stat_p.tile([P, CH], F32, tag="var")
                nc.vector.tensor_tensor(var[:, :ct], mu[:, :ct], mu[:, :ct],
                                        op=ALU.mult)
                nc.vector.tensor_scalar(sq2[:, :ct], sq2[:, :ct], 1.0 / D,
                                        None, ALU.mult)
                nc.vector.tensor_tensor(var[:, :ct], sq2[:, :ct], var[:, :ct],
                                        op=ALU.subtract)
                sd = stat_p.tile([P, CH], F32, tag="sd")
                nc.scalar.activation(sd[:, :ct], var[:, :ct], AF.Sqrt,
                                     bias=epsc[:])
                r0 = stat_p.tile([P, CH], F32, tag="r0")
                nc.vector.reciprocal(r0[:, :ct], sd[:, :ct])
                n0 = stat_p.tile([P, CH], F32, tag="n0")
                nc.vector.tensor_scalar(n0[:, :ct], mu[:, :ct], -1.0, None,
                                        ALU.mult)
                nc.vector.tensor_tensor(n0[:, :ct], n0[:, :ct], r0[:, :ct],
                                        op=ALU.mult)

                for jj in range(ct):
                    t = c0 + jj
                    zn = work_p.tile([P, D], MM_DT, tag="zn")
                    nc.scalar.activation(zn[:], zb[:, jj, :], AF.Identity,
                                         bias=n0[:, jj:jj + 1],
                                         scale=r0[:, jj:jj + 1])
                    znT_ps = psum_p.tile([P, P], MM_DT, tag="tpos", space="PSUM")
                    nc.tensor.transpose(znT_ps[:], zn[:], ident[:])
                    znT = work_p.tile([P, P], MM_DT, tag="znT")
                    nc.vector.tensor_copy(znT[:], znT_ps[:])
                    u12_ps = psum_p.tile([P, 2 * D], F32, tag="mm12", space="PSUM")
                    nc.tensor.matmul(u12_ps[:], lhsT=znT[:], rhs=wcat_sb[:],
                                     start=True, stop=False)
                    nc.tensor.matmul(u12_ps[:], lhsT=ones_row[:], rhs=bcat_sb[:],
                                     start=False, stop=True)
                    nc.scalar.activation(u12buf[:, t, :], u12_ps[:], AF.Relu)

                # LN2/LN3 stats on the chunk's u12 (bf16 in, fp32 reduce)
                u12c = u12buf[:, c0:c0 + ct, :].rearrange(
                    "p t (h d) -> p (t h) d", d=D)
                sqh = stat_p.tile([P, 2 * CH, D], MM_DT, tag="sqh")
                nc.vector.tensor_tensor(sqh[:, :2 * ct, :], u12c, u12c,
                                        op=ALU.mult)
                smh = stat_p.tile([P, 2 * CH], F32, tag="smh")
                nc.vector.tensor_reduce(smh[:, :2 * ct], u12c,
                                        axis=mybir.AxisListType.X, op=ALU.add)
                sqh2 = stat_p.tile([P, 2 * CH], F32, tag="sqh2")
                nc.vector.tensor_reduce(sqh2[:, :2 * ct], sqh[:, :2 * ct, :],
                                        axis=mybir.AxisListType.X, op=ALU.add)
                muh = stat_p.tile([P, 2 * CH], F32, tag="muh")
                nc.vector.tensor_scalar(muh[:, :2 * ct], smh[:, :2 * ct],
                                        1.0 / D, None, ALU.mult)
                varh = stat_p.tile([P, 2 * CH], F32, tag="varh")
                nc.vector.tensor_tensor(varh[:, :2 * ct], muh[:, :2 * ct],
                                        muh[:, :2 * ct], op=ALU.mult)
                nc.vector.tensor_scalar(sqh2[:, :2 * ct], sqh2[:, :2 * ct],
                                        1.0 / D, None, ALU.mult)
                nc.vector.tensor_tensor(varh[:, :2 * ct], sqh2[:, :2 * ct],
                                        varh[:, :2 * ct], op=ALU.subtract)
                sdh = stat_p.tile([P, 2 * CH], F32, tag="sdh")
                nc.scalar.activation(sdh[:, :2 * ct], varh[:, :2 * ct],
                                     AF.Sqrt, bias=epsc[:])
                rh = stat_p.tile([P, 2 * CH], F32, tag="rh")
                nc.vector.reciprocal(rh[:, :2 * ct], sdh[:, :2 * ct])
                nh = stat_p.tile([P, 2 * CH], F32, tag="nh")
                nc.vector.tensor_scalar(nh[:, :2 * ct], muh[:, :2 * ct], -1.0,
                                        None, ALU.mult)
                nc.vector.tensor_tensor(nh[:, :2 * ct], nh[:, :2 * ct],
                                        rh[:, :2 * ct], op=ALU.mult)
                # stash per-tile columns: (t h) layout -> h=0 is u1, h=1 u2
                nc.vector.tensor_copy(
                    r1b[:, c0:c0 + ct],
                    rh[:, 0:2 * ct].rearrange("p (t h) -> p t h", h=2)[:, :, 0])
                nc.vector.tensor_copy(
                    n1b[:, c0:c0 + ct],
                    nh[:, 0:2 * ct].rearrange("p (t h) -> p t h", h=2)[:, :, 0])
                nc.vector.tensor_copy(
                    r2b[:, c0:c0 + ct],
                    rh[:, 0:2 * ct].rearrange("p (t h) -> p t h", h=2)[:, :, 1])
                nc.vector.tensor_copy(
                    n2b[:, c0:c0 + ct],
                    nh[:, 0:2 * ct].rearrange("p (t h) -> p t h", h=2)[:, :, 1])

                # g path for the chunk
                gbuf = gstore_p.tile([P, CH, D], TABLE_DT, tag="gbuf")
                for jj in range(ct):
                    t = c0 + jj
                    h1n = work_p.tile([P, D], MM_DT, tag="h1n")
                    nc.scalar.activation(h1n[:], u12buf[:, t, 0:D], AF.Identity,
                                         bias=n1b[:, t:t + 1],
                                         scale=r1b[:, t:t + 1])
                    h1nT_ps = psum_p.tile([P, P], MM_DT, tag="tpos", space="PSUM")
                    nc.tensor.transpose(h1nT_ps[:], h1n[:], ident[:])
                    h1nT = work_p.tile([P, P], MM_DT, tag="h1nT")
                    nc.vector.tensor_copy(h1nT[:], h1nT_ps[:])
                    g_ps = psum_p.tile([P, D], F32, tag="gmm", space="PSUM")
                    nc.tensor.matmul(g_ps[:], lhsT=h1nT[:], rhs=wbil_sb[:],
                                     start=True, stop=True)
                    nc.vector.tensor_copy(gbuf[:, jj, :], g_ps[:])
                nc.sync.dma_start(
                    g_shard.rearrange("(p j) d -> p j d", p=P)[:, c0:c0 + ct, :],
                    gbuf[:, :ct, :],
                )

            # ---- all-gather g shards into the full table ---------------
            nc.gpsimd.collective_compute(
                "AllGather", ALU.bypass,
                replica_groups=[list(range(NCORES))],
                ins=[g_shard[:]], outs=[g_table[:]],
            )

            # ---- deferred h2 applies (overlap the collective) ----------
            for t in range(SHARD_T):
                nc.scalar.activation(h_sb[:, t, :], u12buf[:, t, D:2 * D],
                                     AF.Identity, bias=n2b[:, t:t + 1],
                                     scale=r2b[:, t:t + 1])

            pa.__exit__(None, None, None)
            pb = tc.tile_pool(name="psumB", bufs=2, space="PSUM")
            psum_p = pb.__enter__()

            # ---- phase B ----------------------------------------------
            for b, (h0, chunks) in enumerate(block_specs):
                lc, lo = divmod(b, LIDX_CHUNK)
                if lo == 0:
                    lrow = lrow_p.tile([1, LIDX_CHUNK * BLK], MM_DT, tag="lrow")
                    nc.scalar.dma_start(lrow[:], lidx_ext[lc][None, :])
                g_src = g_table[HALF:, :] if h0 else g_table[:, :]
                gt = gather_p.tile([P, CB, D], TABLE_DT, tag="gt")
                nc.gpsimd.dma_gather(gt[:], g_src, idx0_sb[:, b, :], BLK, BLK,
                                     D, queue_num=b % NQ)
                lb_ps = psum_p.tile([P, BLK], F32, tag="lb", space="PSUM")
                nc.tensor.matmul(lb_ps[:, 0:512], lhsT=ones_row[:],
                                 rhs=lrow[0:1, lo * BLK:lo * BLK + 512],
                                 start=True, stop=True)
                nc.tensor.matmul(lb_ps[:, 512:1024], lhsT=ones_row[:],
                                 rhs=lrow[0:1, lo * BLK + 512:(lo + 1) * BLK],
                                 start=True, stop=True)
                sT = mask_p.tile([P, BLK], MM_DT, tag="sT")
                nc.vector.tensor_scalar(sT[:], lb_ps[:], iotac[:], None,
                                        ALU.is_equal)
                hsel_ps = psum_p.tile([P, BLK], F32, tag="hsel", space="PSUM")
                for j, ch in enumerate(chunks):
                    nc.tensor.matmul(hsel_ps[:, j * P:(j + 1) * P],
                                     lhsT=sT[:, j * P:(j + 1) * P],
                                     rhs=h_sb[:, ch, :], start=True, stop=True)
                prod = prod_p.tile([P, CB, D], MM_DT, tag="prod")
                nc.vector.tensor_tensor(
                    prod[:].rearrange("p c d -> p (c d)"),
                    gt[:].rearrange("p c d -> p (c d)"),
                    hsel_ps[:], op=ALU.mult)
                nc.vector.tensor_reduce(
                    scorebuf[:, b, :], prod[:], axis=mybir.AxisListType.X,
                    op=ALU.add)

            nc.sync.dma_start(
                out_ext[:],
                scorebuf[:].rearrange("p b c -> p (b c)"),
            )

            pb.__exit__(None, None, None)

    nc.compile()
    return nc


_CACHE = {}
_RUN_KWARGS = {}
LAST_RESULTS = None


def _pack_idx(vals):
    """[nblk, 1024] int16 -> dma_gather SBUF layout [nblk, 128, 64]:
    index k lives at partition k%16, column k//16, replicated into all
    eight 16-partition groups."""
    nblk = vals.shape[0]
    w = vals.reshape(nblk, BLK // 16, 16).transpose(0, 2, 1)   # [nblk,16,S]
    return np.tile(w, (1, 8, 1)).astype(np.int16)


def kernel(**inputs) -> np.ndarray:
    z = np.asarray(inputs["z"], np.float32)
    pot_arcs = np.asarray(inputs["pot_arcs"])
    lin1_w = np.asarray(inputs["lin1_w"], np.float32)
    lin1_b = np.asarray(inputs["lin1_b"], np.float32)
    lin2_w = np.asarray(inputs["lin2_w"], np.float32)
    lin2_b = np.asarray(inputs["lin2_b"], np.float32)
    bil_w = np.asarray(inputs["bil_w"], np.float32)
    bil_b = np.asarray(inputs["bil_b"], np.float32)
    norm_w = np.asarray(inputs["norm_w"], np.float32)
    norm_b = np.asarray(inputs["norm_b"], np.float32)

    if not np.allclose(norm_b, 0.0):
        # general norm_b adds per-node scalar terms; not exercised by this
        # problem's inputs.  Exact numpy fallback keeps kernel() total.
        return _numpy_reference(z, pot_arcs, lin1_w, lin1_b, lin2_w, lin2_b,
                                bil_w, bil_b, norm_w, norm_b)

    w1eff = norm_w[:, None] * lin1_w.T
    b1eff = norm_b @ lin1_w.T + lin1_b
    w2eff = norm_w[:, None] * lin2_w.T
    b2eff = norm_b @ lin2_w.T + lin2_b
    wbil = bil_w[0] * norm_w[None, :]
    wcat = np.concatenate([w1eff, w2eff], axis=1).astype(_np_mdt)
    bcat = np.concatenate([b1eff, b2eff])[None, :].astype(_np_mdt)
    wbil = wbil.astype(_np_mdt)

    zpad = np.zeros((NPAD, D), np.float32)
    zpad[:N] = z
    # row remap: node n -> table row owner*SHARD + (n%SHARD)%P*SHARD_T + (n%SHARD)//P
    noff = np.arange(NPAD, dtype=np.int32) % SHARD
    rowmap = (np.arange(NPAD, dtype=np.int32) // SHARD) * SHARD + \
        (noff % P) * SHARD_T + noff // P

    a0 = pot_arcs[:, 0].astype(np.int32)
    a1 = pot_arcs[:, 1].astype(np.int32)

    # --- assign edges to tail-owner cores; tile by (head-half, tail-chunk) -
    core_of_edge = a1 // SHARD
    per_core = []
    for c in range(NCORES):
        eids = np.where(core_of_edge == c)[0]
        a0c = rowmap[a0[eids]]
        l1 = a1[eids] - c * SHARD
        bucket = (a0c >= HALF).astype(np.int64)
        chunk = l1 // P
        # sort by (bucket, chunk) and cut chunk-pure 128-edge tiles
        order = np.lexsort((chunk, bucket))
        per_core.append((eids, a0c, l1, bucket, chunk, order))

    # tiles per (bucket, chunk) must be uniform across cores (same program)
    ntile_bc = np.zeros((2, SHARD_T), np.int64)
    for c in range(NCORES):
        _, _, _, bucket, chunk, _ = per_core[c]
        for bkt in range(2):
            cnt = np.bincount(chunk[bucket == bkt], minlength=SHARD_T)
            ntile_bc[bkt] = np.maximum(ntile_bc[bkt], -(-cnt // P))
    # tile list: (bucket, chunk) repeated; pad each bucket to multiple of 8
    tiles = []
    for bkt in range(2):
        start = len(tiles)
        for ch in range(SHARD_T):
            tiles += [(bkt, ch)] * int(ntile_bc[bkt, ch])
        while (len(tiles) - start) % CB:
            tiles.append((bkt, 0))
    ntiles = len(tiles)
    nblk = ntiles // CB
    ecpad = ntiles * P

    block_specs = []
    for b in range(nblk):
        bts = tiles[b * CB:(b + 1) * CB]
        assert len({t[0] for t in bts}) == 1
        block_specs.append((bts[0][0], tuple(t[1] for t in bts)))

    # slot ranges per (bucket, chunk)
    tile_start = {}
    pos = 0
    for t in tiles:
        tile_start.setdefault(t, []).append(pos)
        pos += P

    in_maps = []
    gathers = []
    iota_col = np.arange(P, dtype=np.float32).reshape(P, 1)
    nlc = -(-nblk // LIDX_CHUNK)
    for c in range(NCORES):
        eids, a0c, l1, bucket, chunk, order = per_core[c]
        i0 = np.zeros(ecpad, np.int32)
        lidx = np.zeros(ecpad, np.int32)
        gid = np.empty(len(eids), np.int64)
        for bkt in range(2):
            for ch in range(SHARD_T):
                sel = order[(bucket[order] == bkt) & (chunk[order] == ch)]
                starts = tile_start[(bkt, ch)]
                for ti in range(len(starts)):
                    seg = sel[ti * P:(ti + 1) * P]
                    dst = starts[ti] + np.arange(len(seg))
                    i0[dst] = a0c[seg] - (HALF if bkt else 0)
                    lidx[dst] = l1[seg] - ch * P
                    gid[seg] = dst
        gathers.append((eids, gid))
        lidxcat = np.zeros((nlc * LIDX_CHUNK * BLK,), np.float32)
        lidxcat[:ecpad] = lidx.astype(np.float32)
        in_maps.append({
            "z": zpad[c * SHARD:(c + 1) * SHARD].reshape(
                SHARD_T, P, D).transpose(1, 0, 2).reshape(P, SHARD_T * D).copy(),
            "wcat": wcat,
            "bcat": bcat,
            "wbil": wbil,
            "iotac": iota_col,
            "idx0": _pack_idx(i0.astype(np.int16).reshape(nblk, BLK)).transpose(1, 0, 2).copy(),
            "lidx": lidxcat.reshape(nlc, LIDX_CHUNK * BLK).astype(_np_mdt),
        })

    key = tuple(block_specs)
    if key not in _CACHE:
        _CACHE[key] = _build(block_specs)
    nc = _CACHE[key]

    res = run_bass_kernel_spmd(nc, in_maps, list(range(NCORES)), **_RUN_KWARGS)
    global LAST_RESULTS
    LAST_RESULTS = res

    scores = np.empty(E, np.float32)
    for c in range(NCORES):
        out_c = np.asarray(res.results[c]["out"], np.float32).T.ravel()
        eids, gid = gathers[c]
        scores[eids] = out_c[gid]
    return scores + float(bil_b[0])


def _numpy_reference(z, pot_arcs, lin1_w, lin1_b, lin2_w, lin2_b,
                     bil_w, bil_b, norm_w, norm_b):
    def ln(x):
        mu = x.mean(-1, keepdims=True)
        var = x.var(-1, keepdims=True)
        return (x - mu) / np.sqrt(var + 1e-5) * norm_w + norm_b

    zn = ln(z)
    h1 = ln(np.maximum(zn @ lin1_w.T + lin1_b, 0.0))
    h2 = ln(np.maximum(zn @ lin2_w.T + lin2_b, 0.0))
    g = h1 @ bil_w[0]
    a0 = pot_arcs[:, 0].astype(np.int64)
    a1 = pot_arcs[:, 1].astype(np.int64)
    return np.einsum("ed,ed->e", g[a0], h2[a1]) + bil_b[0]


# revision 10
# speedup vs baseline: 1.3348x; 1.1394x over previous
"""ArcDecoder Bass kernel for 8 TRN2 NeuronCores.

Math (norm_w folded into weights host-side; norm_b==0 for this problem,
general case falls back to numpy):
  zn = LN(z); u1 = relu(zn@W1eff+b1); u2 = relu(zn@W2eff+b2)
  h1n = LN(u1); h2n = LN(u2); g = h1n @ Wbil_eff
  score_e = dot(g[a0_e], h2n[a1_e]) + bil_b

Phase A (node tables, sharded): core c computes nodes [c*6272,(c+1)*6272)
  from its own z-shard.  LN stats via batched square+reduce (7-tile
  chunks), normalization applied on the ACT engine (scale/bias APs).
  g-shard -> DRAM -> AllGather into a device-wide table; the h2 applies
  are deferred until after the AllGather is issued so they overlap it.
Phase B (edges assigned to the core owning their TAIL shard):
  * heads: SWDGE dma_gather, 1024 rows/block round-robin over 4 queues;
    all int16 indices preloaded in one DMA.
  * tails: per 1024-edge block one K=1 broadcast matmul pair + one
    1024-wide is_equal builds the one-hot sT; 8 back-to-back selection
    matmuls land in a single [128,1024] PSUM tile.
  * one DVE multiply + one reduce per block -> scores accumulate in
    SBUF; a single DMA stores all scores at the end.  Host adds bil_b
    and inverse-permutes back to input edge order.
"""

import sys

if "/opt/trn_rl_repo" not in sys.path:
    sys.path.insert(0, "/opt/trn_rl_repo")

import numpy as np
import ml_dtypes

import concourse.bass as bass
import concourse.tile as tile
from concourse import bacc, mybir
from concourse.bass_utils import run_bass_kernel_spmd
from concourse.library_config import mlp
from concourse.masks import make_identity

N, D, E = 50000, 128, 500000
NCORES = 8
P = 128
SHARD_T = 49                     # node tiles per core (sharded phase A)
SHARD = SHARD_T * P              # 6272 rows per core
NT = SHARD_T * NCORES            # 392 node tiles total
NPAD = NT * P                    # 50176
CH = 7                           # tiles per phase-A chunk
EC = E // NCORES                 # 62500 edges per core
HALF = 32768                     # int16 index ceiling for dma_gather

F32 = mybir.dt.float32
BF16 = mybir.dt.bfloat16
AF = mybir.ActivationFunctionType
ALU = mybir.AluOpType

TABLE_DT = BF16                  # dtype of g table (gather payload)
MM_DT = BF16                     # dtype of matmul operands
BLK = 1024                       # edges per gather block
CB = BLK // P                    # tiles per gather block
NQ = 4                           # SWDGE queues
LIDX_CHUNK = 4                   # blocks per lidx row load
EPS = 1e-5

_np_tdt = np.float32 if TABLE_DT == F32 else ml_dtypes.bfloat16
_np_mdt = np.float32 if MM_DT == F32 else ml_dtypes.bfloat16


def _build(block_specs):
    """block_specs: list of (head_half, chunk-tuple) per 1024-edge block."""
    nblk = len(block_specs)
    ecpad = nblk * BLK
    S = BLK // 16
    nlc = -(-nblk // LIDX_CHUNK)

    nc = bacc.Bacc("TRN2", target_bir_lowering=False, debug=False,
                   num_devices=NCORES, num_swdge_queues=NQ)

    z_ext = nc.dram_tensor("z", [P, SHARD_T * D], F32, kind="ExternalInput").ap()
    wcat_ext = nc.dram_tensor("wcat", [D, 2 * D], MM_DT, kind="ExternalInput").ap()
    bcat_ext = nc.dram_tensor("bcat", [1, 2 * D], MM_DT, kind="ExternalInput").ap()
    wbil_ext = nc.dram_tensor("wbil", [D, D], MM_DT, kind="ExternalInput").ap()
    iotac_ext = nc.dram_tensor("iotac", [P, 1], F32, kind="ExternalInput").ap()
    idx0_ext = nc.dram_tensor("idx0", [P, nblk, S], mybir.dt.int16,
                              kind="ExternalInput").ap()
    lidx_ext = nc.dram_tensor("lidx", [nlc, LIDX_CHUNK * BLK], MM_DT,
                              kind="ExternalInput").ap()
    out_ext = nc.dram_tensor("out", [P, ecpad // P], F32, kind="ExternalOutput").ap()

    g_shard = nc.dram_tensor("g_shard", [SHARD, D], TABLE_DT).ap()
    g_table = nc.dram_tensor("g_table", [NPAD, D], TABLE_DT, addr_space="Shared").ap()

    with tile.TileContext(nc) as tc:
        with (
            tc.tile_pool(name="const", bufs=1) as const_p,
            tc.tile_pool(name="zload", bufs=2) as zload_p,
            tc.tile_pool(name="stat", bufs=2) as stat_p,
            tc.tile_pool(name="work", bufs=8) as work_p,
            tc.tile_pool(name="gstore", bufs=2) as gstore_p,
            tc.tile_pool(name="gather", bufs=8) as gather_p,
            tc.tile_pool(name="mask", bufs=4) as mask_p,
            tc.tile_pool(name="lrow", bufs=2) as lrow_p,
            tc.tile_pool(name="prod", bufs=3) as prod_p,
        ):
            # ---- constants / preloads ---------------------------------
            nc.gpsimd.load_library(mlp)
            ident = const_p.tile([P, P], MM_DT)
            make_identity(nc, ident[:])
            wcat_sb = const_p.tile([D, 2 * D], MM_DT)
            nc.sync.dma_start(wcat_sb[:], wcat_ext[:])
            bcat_sb = const_p.tile([1, 2 * D], MM_DT)
            nc.sync.dma_start(bcat_sb[:], bcat_ext[:])
            wbil_sb = const_p.tile([D, D], MM_DT)
            nc.sync.dma_start(wbil_sb[:], wbil_ext[:])
            ones_row = const_p.tile([1, P], MM_DT)
            nc.vector.memset(ones_row[:], 1.0)
            epsc = const_p.tile([P, 1], F32)
            nc.vector.memset(epsc[:], EPS)
            iotac = const_p.tile([P, 1], F32)
            nc.sync.dma_start(iotac[:], iotac_ext[:])
            idx0_sb = const_p.tile([P, nblk, S], mybir.dt.int16)
            nc.sync.dma_start(idx0_sb[:], idx0_ext[:])

            h_sb = const_p.tile([P, SHARD_T, D], MM_DT)
            u12buf = const_p.tile([P, SHARD_T, 2 * D], MM_DT)
            scorebuf = const_p.tile([P, nblk, CB], F32)
            # per-tile LN scalars (columns): scale r and bias -mu*r
            r1b = const_p.tile([P, SHARD_T], F32)
            n1b = const_p.tile([P, SHARD_T], F32)
            r2b = const_p.tile([P, SHARD_T], F32)
            n2b = const_p.tile([P, SHARD_T], F32)

            pa = tc.tile_pool(name="psumA", bufs=2, space="PSUM")
            psum_p = pa.__enter__()

            # ---- phase A: z -> u12 -> g (h2 deferred) ------------------
            for c0 in range(0, SHARD_T, CH):
                ct = min(CH, SHARD_T - c0)
                zb = zload_p.tile([P, CH, D], F32, tag="zb")
                nc.sync.dma_start(
                    zb[:, :ct, :],
                    z_ext[:, c0 * D:(c0 + ct) * D].rearrange(
                        "p (j d) -> p j d", d=D),
                )
                # LN1 stats: mean/var via square + reduce
                sq = stat_p.tile([P, CH, D], F32, tag="sq")
                nc.vector.tensor_tensor(sq[:, :ct, :], zb[:, :ct, :],
                                        zb[:, :ct, :], op=ALU.mult)
                sm = stat_p.tile([P, CH], F32, tag="sm")
                nc.vector.tensor_reduce(sm[:, :ct], zb[:, :ct, :],
                                        axis=mybir.AxisListType.X, op=ALU.add)
                sq2 = stat_p.tile([P, CH], F32, tag="sq2")
                nc.vector.tensor_reduce(sq2[:, :ct], sq[:, :ct, :],
                                        axis=mybir.AxisListType.X, op=ALU.add)
                mu = stat_p.tile([P, CH], F32, tag="mu")
                nc.vector.tensor_scalar(mu[:, :ct], sm[:, :ct], 1.0 / D, None,
                                        ALU.mult)
                var = stat_p.tile([P, CH], F32, tag="var")
                nc.vector.tensor_tensor(var[:, :ct], mu[:, :ct], mu[:, :ct],
                                        op=ALU.mult)
                nc.vector.tensor_scalar(sq2[:, :ct], sq2[:, :ct], 1.0 / D,
                                        None, ALU.mult)
                nc.vector.tensor_tensor(var[:, :ct], sq2[:, :ct], var[:, :ct],
                                        op=ALU.subtract)
                sd = stat_p.tile([P, CH], F32, tag="sd")
                nc.scalar.activation(sd[:, :ct], var[:, :ct], AF.Sqrt,
                                     bias=epsc[:])
                r0 = stat_p.tile([P, CH], F32, tag="r0")
                nc.vector.reciprocal(r0[:, :ct], sd[:, :ct])
                n0 = stat_p.tile([P, CH], F32, tag="n0")
                nc.vector.tensor_scalar(n0[:, :ct], mu[:, :ct], -1.0, None,
                                        ALU.mult)
                nc.vector.tensor_tensor(n0[:, :ct], n0[:, :ct], r0[:, :ct],
                                        op=ALU.mult)

                for jj in range(ct):
                    t = c0 + jj
                    zn = work_p.tile([P, D], MM_DT, tag="zn")
                    nc.scalar.activation(zn[:], zb[:, jj, :], AF.Identity,
                                         bias=n0[:, jj:jj + 1],
                                         scale=r0[:, jj:jj + 1])
                    znT_ps = psum_p.tile([P, P], MM_DT, tag="tpos", space="PSUM")
                    nc.tensor.transpose(znT_ps[:], zn[:], ident[:])
                    znT = work_p.tile([P, P], MM_DT, tag="znT")
                    nc.vector.tensor_copy(znT[:], znT_ps[:])
                    u12_ps = psum_p.tile([P, 2 * D], F32, tag="mm12", space="PSUM")
                    nc.tensor.matmul(u12_ps[:], lhsT=znT[:], rhs=wcat_sb[:],
                                     start=True, stop=False)
                    nc.tensor.matmul(u12_ps[:], lhsT=ones_row[:], rhs=bcat_sb[:],
                                     start=False, stop=True)
                    nc.scalar.activation(u12buf[:, t, :], u12_ps[:], AF.Relu)

                # LN2/LN3 stats on the chunk's u12 (bf16 in, fp32 reduce)
                u12c = u12buf[:, c0:c0 + ct, :].rearrange(
                    "p t (h d) -> p (t h) d", d=D)
                sqh = stat_p.tile([P, 2 * CH, D], MM_DT, tag="sqh")
                nc.vector.tensor_tensor(sqh[:, :2 * ct, :], u12c, u12c,
                                        op=ALU.mult)
                smh = stat_p.tile([P, 2 * CH], F32, tag="smh")
                nc.vector.tensor_reduce(smh[:, :2 * ct], u12c,
                                        axis=mybir.AxisListType.X, op=ALU.add)
                sqh2 = stat_p.tile([P, 2 * CH], F32, tag="sqh2")
                nc.vector.tensor_reduce(sqh2[:, :2 * ct], sqh[:, :2 * ct, :],
                                        axis=mybir.AxisListType.X, op=ALU.add)
                muh = stat_p.tile([P, 2 * CH], F32, tag="muh")
                nc.vector.tensor_scalar(muh[:, :2 * ct], smh[:, :2 * ct],
                                        1.0 / D, None, ALU.mult)
                varh = stat_p.tile([P, 2 * CH], F32, tag="varh")
                nc.vector.tensor_tensor(varh[:, :2 * ct], muh[:, :2 * ct],
                                        muh[:, :2 * ct], op=ALU.mult)
                nc.vector.tensor_scalar(sqh2[:, :2 * ct], sqh2[:, :2 * ct],
                                        1.0 / D, None, ALU.mult)
                nc.vector.tensor_tensor(varh[:, :2 * ct], sqh2[:, :2 * ct],
                                        varh[:, :2 * ct], op=ALU.subtract)
                sdh = stat_p.tile([P, 2 * CH], F32, tag="sdh")
                nc.scalar.activation(sdh[:, :2 * ct], varh[:, :2 * ct],
                                     AF.Sqrt, bias=epsc[:])
                rh = stat_p.tile([P, 2 * CH], F32, tag="rh")
                nc.vector.reciprocal(rh[:, :2 * ct], sdh[:, :2 * ct])
                nh = stat_p.tile([P, 2 * CH], F32, tag="nh")
                nc.vector.tensor_scalar(nh[:, :2 * ct], muh[:, :2 * ct], -1.0,
                                        None, ALU.mult)
                nc.vector.tensor_tensor(nh[:, :2 * ct], nh[:, :2 * ct],
                                        rh[:, :2 * ct], op=ALU.mult)
                # stash per-tile columns: (t h) layout -> h=0 is u1, h=1 u2
                nc.vector.tensor_copy(
                    r1b[:, c0:c0 + ct],
                    rh[:, 0:2 * ct].rearrange("p (t h) -> p t h", h=2)[:, :, 0])
                nc.vector.tensor_copy(
                    n1b[:, c0:c0 + ct],
                    nh[:, 0:2 * ct].rearrange("p (t h) -> p t h", h=2)[:, :, 0])
                nc.vector.tensor_copy(
                    r2b[:, c0:c0 + ct],
                    rh[:, 0:2 * ct].rearrange("p (t h) -> p t h", h=2)[:, :, 1])
                nc.vector.tensor_copy(
                    n2b[:, c0:c0 + ct],
                    nh[:, 0:2 * ct].rearrange("p (t h) -> p t h", h=2)[:, :, 1])

                # g path for the chunk
                gbuf = gstore_p.tile([P, CH, D], TABLE_DT, tag="gbuf")
                for jj in range(ct):
                    t = c0 + jj
                    h1n = work_p.tile([P, D], MM_DT, tag="h1n")
                    nc.scalar.activation(h1n[:], u12buf[:, t, 0:D], AF.Identity,
                                         bias=n1b[:, t:t + 1],
                                         scale=r1b[:, t:t + 1])
                    h1nT_ps = psum_p.tile([P, P], MM_DT, tag="tpos", space="PSUM")
                    nc.tensor.transpose(h1nT_ps[:], h1n[:], ident[:])
                    h1nT = work_p.tile([P, P], MM_DT, tag="h1nT")
                    nc.vector.tensor_copy(h1nT[:], h1nT_ps[:])
                    g_ps = psum_p.tile([P, D], F32, tag="gmm", space="PSUM")
                    nc.tensor.matmul(g_ps[:], lhsT=h1nT[:], rhs=wbil_sb[:],
                                     start=True, stop=True)
                    nc.vector.tensor_copy(gbuf[:, jj, :], g_ps[:])
                nc.sync.dma_start(
                    g_shard.rearrange("(p j) d -> p j d", p=P)[:, c0:c0 + ct, :],
                    gbuf[:, :ct, :],
                )

            # ---- all-gather g shards into the full table ---------------
            nc.gpsimd.collective_compute(
                "AllGather", ALU.bypass,
                replica_groups=[list(range(NCORES))],
                ins=[g_shard[:]], outs=[g_table[:]],
            )

            # ---- deferred h2 applies (overlap the collective) ----------
            for t in range(SHARD_T):
                nc.scalar.activation(h_sb[:, t, :], u12buf[:, t, D:2 * D],
                                     AF.Identity, bias=n2b[:, t:t + 1],
                                     scale=r2b[:, t:t + 1])

            pa.__exit__(None, None, None)
            pb = tc.tile_pool(name="psumB", bufs=2, space="PSUM")
            psum_p = pb.__enter__()

            # ---- phase B ----------------------------------------------
            for b, (h0, chunks) in enumerate(block_specs):
                lc, lo = divmod(b, LIDX_CHUNK)
                if lo == 0:
                    lrow = lrow_p.tile([1, LIDX_CHUNK * BLK], MM_DT, tag="lrow")
                    nc.sync.dma_start(lrow[:], lidx_ext[lc][None, :])
                g_src = g_table[HALF:, :] if h0 else g_table[:, :]
                gt = gather_p.tile([P, CB, D], TABLE_DT, tag="gt")
                nc.gpsimd.dma_gather(gt[:], g_src, idx0_sb[:, b, :], BLK, BLK,
                                     D, queue_num=b % NQ)
                lb_ps = psum_p.tile([P, BLK], F32, tag="lb", space="PSUM")
                nc.tensor.matmul(lb_ps[:, 0:512], lhsT=ones_row[:],
                                 rhs=lrow[0:1, lo * BLK:lo * BLK + 512],
                                 start=True, stop=True)
                nc.tensor.matmul(lb_ps[:, 512:1024], lhsT=ones_row[:],
                                 rhs=lrow[0:1, lo * BLK + 512:(lo + 1) * BLK],
                                 start=True, stop=True)
                sT = mask_p.tile([P, BLK], MM_DT, tag="sT")
                nc.vector.tensor_scalar(sT[:], lb_ps[:], iotac[:], None,
                                        ALU.is_equal)
                hsel_ps = psum_p.tile([P, BLK], F32, tag="hsel", space="PSUM")
                for j, ch in enumerate(chunks):
                    nc.tensor.matmul(hsel_ps[:, j * P:(j + 1) * P],
                                     lhsT=sT[:, j * P:(j + 1) * P],
                                     rhs=h_sb[:, ch, :], start=True, stop=True)
                prod = prod_p.tile([P, CB, D], MM_DT, tag="prod")
                nc.vector.tensor_tensor(
                    prod[:].rearrange("p c d -> p (c d)"),
                    gt[:].rearrange("p c d -> p (c d)"),
                    hsel_ps[:], op=ALU.mult)
                nc.vector.tensor_reduce(
                    scorebuf[:, b, :], prod[:], axis=mybir.AxisListType.X,
                    op=ALU.add)

            nc.sync.dma_start(
                out_ext[:],
                scorebuf[:].rearrange("p b c -> p (b c)"),
            )

            pb.__exit__(None, None, None)

    nc.compile()
    return nc


_CACHE = {}
_RUN_KWARGS = {}
LAST_RESULTS = None


def _pack_idx(vals):
    """[nblk, 1024] int16 -> dma_gather SBUF layout [nblk, 128, 64]:
    index k lives at partition k%16, column k//16, replicated into all
    eight 16-partition groups."""
    nblk = vals.shape[0]
    w = vals.reshape(nblk, BLK // 16, 16).transpose(0, 2, 1)   # [nblk,16,S]
    return np.tile(w, (1, 8, 1)).astype(np.int16)


def kernel(**inputs) -> np.ndarray:
    z = np.asarray(inputs["z"], np.float32)
    pot_arcs = np.asarray(inputs["pot_arcs"])
    lin1_w = np.asarray(inputs["lin1_w"], np.float32)
    lin1_b = np.asarray(inputs["lin1_b"], np.float32)
    lin2_w = np.asarray(inputs["lin2_w"], np.float32)
    lin2_b = np.asarray(inputs["lin2_b"], np.float32)
    bil_w = np.asarray(inputs["bil_w"], np.float32)
    bil_b = np.asarray(inputs["bil_b"], np.float32)
    norm_w = np.asarray(inputs["norm_w"], np.float32)
    norm_b = np.asarray(inputs["norm_b"], np.float32)

    if not np.allclose(norm_b, 0.0):
        # general norm_b adds per-node scalar terms; not exercised by this
        # problem's inputs.  Exact numpy fallback keeps kernel() total.
        return _numpy_reference(z, pot_arcs, lin1_w, lin1_b, lin2_w, lin2_b,
                                bil_w, bil_b, norm_w, norm_b)

    w1eff = norm_w[:, None] * lin1_w.T
    b1eff = norm_b @ lin1_w.T + lin1_b
    w2eff = norm_w[:, None] * lin2_w.T
    b2eff = norm_b @ lin2_w.T + lin2_b
    wbil = bil_w[0] * norm_w[None, :]
    wcat = np.concatenate([w1eff, w2eff], axis=1).astype(_np_mdt)
    bcat = np.concatenate([b1eff, b2eff])[None, :].astype(_np_mdt)
    wbil = wbil.astype(_np_mdt)

    zpad = np.zeros((NPAD, D), np.float32)
    zpad[:N] = z
    # row remap: node n -> table row owner*SHARD + (n%SHARD)%P*SHARD_T + (n%SHARD)//P
    noff = np.arange(NPAD, dtype=np.int32) % SHARD
    rowmap = (np.arange(NPAD, dtype=np.int32) // SHARD) * SHARD + \
        (noff % P) * SHARD_T + noff // P

    a0 = pot_arcs[:, 0].astype(np.int32)
    a1 = pot_arcs[:, 1].astype(np.int32)

    # --- assign edges to tail-owner cores; tile by (head-half, tail-chunk) -
    core_of_edge = a1 // SHARD
    per_core = []
    for c in range(NCORES):
        eids = np.where(core_of_edge == c)[0]
        a0c = rowmap[a0[eids]]
        l1 = a1[eids] - c * SHARD
        bucket = (a0c >= HALF).astype(np.int64)
        chunk = l1 // P
        # sort by (bucket, chunk) and cut chunk-pure 128-edge tiles
        order = np.lexsort((chunk, bucket))
        per_core.append((eids, a0c, l1, bucket, chunk, order))

    # tiles per (bucket, chunk) must be uniform across cores (same program)
    ntile_bc = np.zeros((2, SHARD_T), np.int64)
    for c in range(NCORES):
        _, _, _, bucket, chunk, _ = per_core[c]
        for bkt in range(2):
            cnt = np.bincount(chunk[bucket == bkt], minlength=SHARD_T)
            ntile_bc[bkt] = np.maximum(ntile_bc[bkt], -(-cnt // P))
    # tile list: (bucket, chunk) repeated; pad each bucket to multiple of 8
    tiles = []
    for bkt in range(2):
        start = len(tiles)
        for ch in range(SHARD_T):
            tiles += [(bkt, ch)] * int(ntile_bc[bkt, ch])
        while (len(tiles) - start) % CB:
            tiles.append((bkt, 0))
    ntiles = len(tiles)
    nblk = ntiles // CB
    ecpad = ntiles * P

    block_specs = []
    for b in range(nblk):
        bts = tiles[b * CB:(b + 1) * CB]
        assert len({t[0] for t in bts}) == 1
        block_specs.append((bts[0][0], tuple(t[1] for t in bts)))

    # slot ranges per (bucket, chunk)
    tile_start = {}
    pos = 0
    for t in tiles:
        tile_start.setdefault(t, []).append(pos)
        pos += P

    in_maps = []
    gathers = []
    iota_col = np.arange(P, dtype=np.float32).reshape(P, 1)
    nlc = -(-nblk // LIDX_CHUNK)
    for c in range(NCORES):
        eids, a0c, l1, bucket, chunk, order = per_core[c]
        i0 = np.zeros(ecpad, np.int32)
        lidx = np.zeros(ecpad, np.int32)
        gid = np.empty(len(eids), np.int64)
        for bkt in range(2):
            for ch in range(SHARD_T):
                sel = order[(bucket[order] == bkt) & (chunk[order] == ch)]
                starts = tile_start[(bkt, ch)]
                for ti in range(len(starts)):
                    seg = sel[ti * P:(ti + 1) * P]
                    dst = starts[ti] + np.arange(len(seg))
                    i0[dst] = a0c[seg] - (HALF if bkt else 0)
                    lidx[dst] = l1[seg] - ch * P
                    gid[seg] = dst
        gathers.append((eids, gid))
        lidxcat = np.zeros((nlc * LIDX_CHUNK * BLK,), np.float32)
        lidxcat[:ecpad] = lidx.astype(np.float32)
        in_maps.append({
            "z": zpad[c * SHARD:(c + 1) * SHARD].reshape(
                SHARD_T, P, D).transpose(1, 0, 2).reshape(P, SHARD_T * D).copy(),
            "wcat": wcat,
            "bcat": bcat,
            "wbil": wbil,
            "iotac": iota_col,
            "idx0": _pack_idx(i0.astype(np.int16).reshape(nblk, BLK)).transpose(1, 0, 2).copy(),
            "lidx": lidxcat.reshape(nlc, LIDX_CHUNK * BLK).astype(_np_mdt),
        })

    key = tuple(block_specs)
    if key not in _CACHE:
        _CACHE[key] = _build(block_specs)
    nc = _CACHE[key]

    res = run_bass_kernel_spmd(nc, in_maps, list(range(NCORES)), **_RUN_KWARGS)
    global LAST_RESULTS
    LAST_RESULTS = res

    scores = np.empty(E, np.float32)
    for c in range(NCORES):
        out_c = np.asarray(res.results[c]["out"], np.float32).T.ravel()
        eids, gid = gathers[c]
        scores[eids] = out_c[gid]
    return scores + float(bil_b[0])


def _numpy_reference(z, pot_arcs, lin1_w, lin1_b, lin2_w, lin2_b,
                     bil_w, bil_b, norm_w, norm_b):
    def ln(x):
        mu = x.mean(-1, keepdims=True)
        var = x.var(-1, keepdims=True)
        return (x - mu) / np.sqrt(var + 1e-5) * norm_w + norm_b

    zn = ln(z)
    h1 = ln(np.maximum(zn @ lin1_w.T + lin1_b, 0.0))
    h2 = ln(np.maximum(zn @ lin2_w.T + lin2_b, 0.0))
    g = h1 @ bil_w[0]
    a0 = pot_arcs[:, 0].astype(np.int64)
    a1 = pot_arcs[:, 1].astype(np.int64)
    return np.einsum("ed,ed->e", g[a0], h2[a1]) + bil_b[0]


# revision 11
# speedup vs baseline: 1.3406x; 1.0044x over previous
"""ArcDecoder Bass kernel for 8 TRN2 NeuronCores.

Math (norm_w folded into weights host-side; norm_b==0 for this problem,
general case falls back to numpy):
  zn = LN(z); u1 = relu(zn@W1eff+b1); u2 = relu(zn@W2eff+b2)
  h1n = LN(u1); h2n = LN(u2); g = h1n @ Wbil_eff
  score_e = dot(g[a0_e], h2n[a1_e]) + bil_b

Phase A (node tables, sharded): core c computes nodes [c*6272,(c+1)*6272)
  from its own z-shard.  LN stats batched per 7-tile chunk (square +
  reduce on DVE), normalization applied on the ACT engine via per-tile
  scale/bias column APs.  g rows are stored p-major (table row =
  owner*6272 + p*49 + t) so every DMA is partition-contiguous, then
  AllGathered into a device-wide table; h2 applies are deferred past the
  collective so they overlap it.
Phase B (edges assigned to the core owning their TAIL shard):
  * heads: SWDGE dma_gather of 256B bf16 g-rows, 1024/block, round-robin
    over 4 queues; all int16 indices preloaded in one p-major DMA.
  * tails: per block, a K=1 ones-matmul pair broadcasts the tail offsets
    into PSUM, one 1024-wide is_equal vs an iota column builds the
    one-hot sT, and 8 back-to-back selection matmuls land the selected
    h2 rows in a single 2-bank [128,1024] PSUM tile.
  * one DVE multiply (bf16 x PSUM) + one reduce per block; scores
    accumulate in SBUF and a single partition-contiguous DMA stores
    them.  Host adds bil_b and inverse-permutes to input edge order.

All DMAs are partition-contiguous by construction (pre-transposed z
input, p-major g-table row mapping, p-major idx packing, [P, X] output)
-- transposing access patterns otherwise degrade to 4-256B descriptors
on the static-DMA queue and serialize the kernel.
"""

import sys

if "/opt/trn_rl_repo" not in sys.path:
    sys.path.insert(0, "/opt/trn_rl_repo")

import numpy as np
import ml_dtypes

import concourse.bass as bass
import concourse.tile as tile
from concourse import bacc, mybir
from concourse.bass_utils import run_bass_kernel_spmd
from concourse.library_config import mlp
from concourse.masks import make_identity

N, D, E = 50000, 128, 500000
NCORES = 8
P = 128
SHARD_T = 49                     # node tiles per core (sharded phase A)
SHARD = SHARD_T * P              # 6272 rows per core
NT = SHARD_T * NCORES            # 392 node tiles total
NPAD = NT * P                    # 50176
CH = 7                           # tiles per phase-A chunk
EC = E // NCORES                 # 62500 edges per core
HALF = 32768                     # int16 index ceiling for dma_gather

F32 = mybir.dt.float32
BF16 = mybir.dt.bfloat16
AF = mybir.ActivationFunctionType
ALU = mybir.AluOpType

TABLE_DT = BF16                  # dtype of g table (gather payload)
MM_DT = BF16                     # dtype of matmul operands
BLK = 1024                       # edges per gather block
CB = BLK // P                    # tiles per gather block
NQ = 4                           # SWDGE queues
LIDX_CHUNK = 4                   # blocks per lidx row load
EPS = 1e-5

_np_tdt = np.float32 if TABLE_DT == F32 else ml_dtypes.bfloat16
_np_mdt = np.float32 if MM_DT == F32 else ml_dtypes.bfloat16


def _build(block_specs):
    """block_specs: list of (head_half, chunk-tuple) per 1024-edge block."""
    nblk = len(block_specs)
    ecpad = nblk * BLK
    S = BLK // 16
    nlc = -(-nblk // LIDX_CHUNK)

    nc = bacc.Bacc("TRN2", target_bir_lowering=False, debug=False,
                   num_devices=NCORES, num_swdge_queues=NQ)

    z_ext = nc.dram_tensor("z", [P, SHARD_T * D], F32, kind="ExternalInput").ap()
    wcat_ext = nc.dram_tensor("wcat", [D, 2 * D], MM_DT, kind="ExternalInput").ap()
    bcat_ext = nc.dram_tensor("bcat", [1, 2 * D], MM_DT, kind="ExternalInput").ap()
    wbil_ext = nc.dram_tensor("wbil", [D, D], MM_DT, kind="ExternalInput").ap()
    iotac_ext = nc.dram_tensor("iotac", [P, 1], F32, kind="ExternalInput").ap()
    idx0_ext = nc.dram_tensor("idx0", [P, nblk, S], mybir.dt.int16,
                              kind="ExternalInput").ap()
    lidx_ext = nc.dram_tensor("lidx", [nlc, LIDX_CHUNK * BLK], MM_DT,
                              kind="ExternalInput").ap()
    out_ext = nc.dram_tensor("out", [P, ecpad // P], F32, kind="ExternalOutput").ap()

    g_shard = nc.dram_tensor("g_shard", [SHARD, D], TABLE_DT).ap()
    g_table = nc.dram_tensor("g_table", [NPAD, D], TABLE_DT, addr_space="Shared").ap()

    with tile.TileContext(nc) as tc:
        with (
            tc.tile_pool(name="const", bufs=1) as const_p,
            tc.tile_pool(name="zload", bufs=2) as zload_p,
            tc.tile_pool(name="stat", bufs=2) as stat_p,
            tc.tile_pool(name="work", bufs=8) as work_p,
            tc.tile_pool(name="gstore", bufs=2) as gstore_p,
            tc.tile_pool(name="gather", bufs=8) as gather_p,
            tc.tile_pool(name="mask", bufs=4) as mask_p,
            tc.tile_pool(name="lrow", bufs=2) as lrow_p,
            tc.tile_pool(name="prod", bufs=3) as prod_p,
        ):
            # ---- constants / preloads ---------------------------------
            nc.gpsimd.load_library(mlp)
            ident = const_p.tile([P, P], MM_DT)
            make_identity(nc, ident[:])
            wcat_sb = const_p.tile([D, 2 * D], MM_DT)
            nc.sync.dma_start(wcat_sb[:], wcat_ext[:])
            bcat_sb = const_p.tile([1, 2 * D], MM_DT)
            nc.sync.dma_start(bcat_sb[:], bcat_ext[:])
            wbil_sb = const_p.tile([D, D], MM_DT)
            nc.sync.dma_start(wbil_sb[:], wbil_ext[:])
            ones_row = const_p.tile([1, P], MM_DT)
            nc.vector.memset(ones_row[:], 1.0)
            epsc = const_p.tile([P, 1], F32)
            nc.vector.memset(epsc[:], EPS)
            iotac = const_p.tile([P, 1], F32)
            nc.sync.dma_start(iotac[:], iotac_ext[:])
            idx0_sb = const_p.tile([P, nblk, S], mybir.dt.int16)
            nc.sync.dma_start(idx0_sb[:], idx0_ext[:])

            h_sb = const_p.tile([P, SHARD_T, D], MM_DT)
            u12buf = const_p.tile([P, SHARD_T, 2 * D], MM_DT)
            scorebuf = const_p.tile([P, nblk, CB], F32)
            # per-tile LN scalars (columns): scale r and bias -mu*r
            r1b = const_p.tile([P, SHARD_T], F32)
            n1b = const_p.tile([P, SHARD_T], F32)
            r2b = const_p.tile([P, SHARD_T], F32)
            n2b = const_p.tile([P, SHARD_T], F32)

            pa = tc.tile_pool(name="psumA", bufs=2, space="PSUM")
            psum_p = pa.__enter__()

            # ---- phase A: z -> u12 -> g (h2 deferred) ------------------
            for c0 in range(0, SHARD_T, CH):
                ct = min(CH, SHARD_T - c0)
                zb = zload_p.tile([P, CH, D], F32, tag="zb")
                nc.sync.dma_start(
                    zb[:, :ct, :],
                    z_ext[:, c0 * D:(c0 + ct) * D].rearrange(
                        "p (j d) -> p j d", d=D),
                )
                # LN1 stats: mean/var via square + reduce
                sq = stat_p.tile([P, CH, D], F32, tag="sq")
                nc.vector.tensor_tensor(sq[:, :ct, :], zb[:, :ct, :],
                                        zb[:, :ct, :], op=ALU.mult)
                sm = stat_p.tile([P, CH], F32, tag="sm")
                nc.vector.tensor_reduce(sm[:, :ct], zb[:, :ct, :],
                                        axis=mybir.AxisListType.X, op=ALU.add)
                sq2 = stat_p.tile([P, CH], F32, tag="sq2")
                nc.vector.tensor_reduce(sq2[:, :ct], sq[:, :ct, :],
                                        axis=mybir.AxisListType.X, op=ALU.add)
                mu = stat_p.tile([P, CH], F32, tag="mu")
                nc.vector.tensor_scalar(mu[:, :ct], sm[:, :ct], 1.0 / D, None,
                                        ALU.mult)
                var = stat_p.tile([P, CH], F32, tag="var")
                nc.vector.tensor_tensor(var[:, :ct], mu[:, :ct], mu[:, :ct],
                                        op=ALU.mult)
                nc.vector.tensor_scalar(sq2[:, :ct], sq2[:, :ct], 1.0 / D,
                                        None, ALU.mult)
                nc.vector.tensor_tensor(var[:, :ct], sq2[:, :ct], var[:, :ct],
                                        op=ALU.subtract)
                sd = stat_p.tile([P, CH], F32, tag="sd")
                nc.scalar.activation(sd[:, :ct], var[:, :ct], AF.Sqrt,
                                     bias=epsc[:])
                r0 = stat_p.tile([P, CH], F32, tag="r0")
                nc.vector.reciprocal(r0[:, :ct], sd[:, :ct])
                n0 = stat_p.tile([P, CH], F32, tag="n0")
                nc.vector.tensor_scalar(n0[:, :ct], mu[:, :ct], -1.0, None,
                                        ALU.mult)
                nc.vector.tensor_tensor(n0[:, :ct], n0[:, :ct], r0[:, :ct],
                                        op=ALU.mult)

                for jj in range(ct):
                    t = c0 + jj
                    zn = work_p.tile([P, D], MM_DT, tag="zn")
                    nc.scalar.activation(zn[:], zb[:, jj, :], AF.Identity,
                                         bias=n0[:, jj:jj + 1],
                                         scale=r0[:, jj:jj + 1])
                    znT_ps = psum_p.tile([P, P], MM_DT, tag="tpos", space="PSUM")
                    nc.tensor.transpose(znT_ps[:], zn[:], ident[:])
                    znT = work_p.tile([P, P], MM_DT, tag="znT")
                    nc.vector.tensor_copy(znT[:], znT_ps[:])
                    u12_ps = psum_p.tile([P, 2 * D], F32, tag="mm12", space="PSUM")
                    nc.tensor.matmul(u12_ps[:], lhsT=znT[:], rhs=wcat_sb[:],
                                     start=True, stop=False)
                    nc.tensor.matmul(u12_ps[:], lhsT=ones_row[:], rhs=bcat_sb[:],
                                     start=False, stop=True)
                    nc.scalar.activation(u12buf[:, t, :], u12_ps[:], AF.Relu)

                # LN2/LN3 stats on the chunk's u12 (bf16 in, fp32 reduce)
                u12c = u12buf[:, c0:c0 + ct, :].rearrange(
                    "p t (h d) -> p (t h) d", d=D)
                sqh = stat_p.tile([P, 2 * CH, D], MM_DT, tag="sqh")
                nc.vector.tensor_tensor(sqh[:, :2 * ct, :], u12c, u12c,
                                        op=ALU.mult)
                smh = stat_p.tile([P, 2 * CH], F32, tag="smh")
                nc.vector.tensor_reduce(smh[:, :2 * ct], u12c,
                                        axis=mybir.AxisListType.X, op=ALU.add)
                sqh2 = stat_p.tile([P, 2 * CH], F32, tag="sqh2")
                nc.vector.tensor_reduce(sqh2[:, :2 * ct], sqh[:, :2 * ct, :],
                                        axis=mybir.AxisListType.X, op=ALU.add)
                muh = stat_p.tile([P, 2 * CH], F32, tag="muh")
                nc.vector.tensor_scalar(muh[:, :2 * ct], smh[:, :2 * ct],
                                        1.0 / D, None, ALU.mult)
                varh = stat_p.tile([P, 2 * CH], F32, tag="varh")
                nc.vector.tensor_tensor(varh[:, :2 * ct], muh[:, :2 * ct],
                                        muh[:, :2 * ct], op=ALU.mult)
                nc.vector.tensor_scalar(sqh2[:, :2 * ct], sqh2[:, :2 * ct],
                                        1.0 / D, None, ALU.mult)
                nc.vector.tensor_tensor(varh[:, :2 * ct], sqh2[:, :2 * ct],
                                        varh[:, :2 * ct], op=ALU.subtract)
                sdh = stat_p.tile([P, 2 * CH], F32, tag="sdh")
                nc.scalar.activation(sdh[:, :2 * ct], varh[:, :2 * ct],
                                     AF.Sqrt, bias=epsc[:])
                rh = stat_p.tile([P, 2 * CH], F32, tag="rh")
                nc.vector.reciprocal(rh[:, :2 * ct], sdh[:, :2 * ct])
                nh = stat_p.tile([P, 2 * CH], F32, tag="nh")
                nc.vector.tensor_scalar(nh[:, :2 * ct], muh[:, :2 * ct], -1.0,
                                        None, ALU.mult)
                nc.vector.tensor_tensor(nh[:, :2 * ct], nh[:, :2 * ct],
                                        rh[:, :2 * ct], op=ALU.mult)
                # stash per-tile columns: (t h) layout -> h=0 is u1, h=1 u2
                nc.vector.tensor_copy(
                    r1b[:, c0:c0 + ct],
                    rh[:, 0:2 * ct].rearrange("p (t h) -> p t h", h=2)[:, :, 0])
                nc.vector.tensor_copy(
                    n1b[:, c0:c0 + ct],
                    nh[:, 0:2 * ct].rearrange("p (t h) -> p t h", h=2)[:, :, 0])
                nc.vector.tensor_copy(
                    r2b[:, c0:c0 + ct],
                    rh[:, 0:2 * ct].rearrange("p (t h) -> p t h", h=2)[:, :, 1])
                nc.vector.tensor_copy(
                    n2b[:, c0:c0 + ct],
                    nh[:, 0:2 * ct].rearrange("p (t h) -> p t h", h=2)[:, :, 1])

                # g path for the chunk
                gbuf = gstore_p.tile([P, CH, D], TABLE_DT, tag="gbuf")
                for jj in range(ct):
                    t = c0 + jj
                    h1n = work_p.tile([P, D], MM_DT, tag="h1n")
                    nc.scalar.activation(h1n[:], u12buf[:, t, 0:D], AF.Identity,
                                         bias=n1b[:, t:t + 1],
                                         scale=r1b[:, t:t + 1])
                    h1nT_ps = psum_p.tile([P, P], MM_DT, tag="tpos", space="PSUM")
                    nc.tensor.transpose(h1nT_ps[:], h1n[:], ident[:])
                    h1nT = work_p.tile([P, P], MM_DT, tag="h1nT")
                    nc.vector.tensor_copy(h1nT[:], h1nT_ps[:])
                    g_ps = psum_p.tile([P, D], F32, tag="gmm", space="PSUM")
                    nc.tensor.matmul(g_ps[:], lhsT=h1nT[:], rhs=wbil_sb[:],
                                     start=True, stop=True)
                    nc.vector.tensor_copy(gbuf[:, jj, :], g_ps[:])
                nc.sync.dma_start(
                    g_shard.rearrange("(p j) d -> p j d", p=P)[:, c0:c0 + ct, :],
                    gbuf[:, :ct, :],
                )

            # ---- all-gather g shards into the full table ---------------
            nc.gpsimd.collective_compute(
                "AllGather", ALU.bypass,
                replica_groups=[list(range(NCORES))],
                ins=[g_shard[:]], outs=[g_table[:]],
            )

            # ---- deferred h2 applies (overlap the collective) ----------
            for t in range(SHARD_T):
                nc.scalar.activation(h_sb[:, t, :], u12buf[:, t, D:2 * D],
                                     AF.Identity, bias=n2b[:, t:t + 1],
                                     scale=r2b[:, t:t + 1])

            pa.__exit__(None, None, None)
            pb = tc.tile_pool(name="psumB", bufs=2, space="PSUM")
            psum_p = pb.__enter__()

            # ---- phase B ----------------------------------------------
            for b, (h0, chunks) in enumerate(block_specs):
                lc, lo = divmod(b, LIDX_CHUNK)
                if lo == 0:
                    lrow = lrow_p.tile([1, LIDX_CHUNK * BLK], MM_DT, tag="lrow")
                    nc.sync.dma_start(lrow[:], lidx_ext[lc][None, :])
                g_src = g_table[HALF:, :] if h0 else g_table[:, :]
                gt = gather_p.tile([P, CB, D], TABLE_DT, tag="gt")
                nc.gpsimd.dma_gather(gt[:], g_src, idx0_sb[:, b, :], BLK, BLK,
                                     D, queue_num=b % NQ)
                lb_ps = psum_p.tile([P, BLK], F32, tag="lb", space="PSUM")
                nc.tensor.matmul(lb_ps[:, 0:512], lhsT=ones_row[:],
                                 rhs=lrow[0:1, lo * BLK:lo * BLK + 512],
                                 start=True, stop=True)
                nc.tensor.matmul(lb_ps[:, 512:1024], lhsT=ones_row[:],
                                 rhs=lrow[0:1, lo * BLK + 512:(lo + 1) * BLK],
                                 start=True, stop=True)
                sT = mask_p.tile([P, BLK], MM_DT, tag="sT")
                nc.vector.tensor_scalar(sT[:], lb_ps[:], iotac[:], None,
                                        ALU.is_equal)
                hsel_ps = psum_p.tile([P, BLK], F32, tag="hsel", space="PSUM")
                for j, ch in enumerate(chunks):
                    nc.tensor.matmul(hsel_ps[:, j * P:(j + 1) * P],
                                     lhsT=sT[:, j * P:(j + 1) * P],
                                     rhs=h_sb[:, ch, :], start=True, stop=True)
                prod = prod_p.tile([P, CB, D], MM_DT, tag="prod")
                nc.vector.tensor_tensor(
                    prod[:].rearrange("p c d -> p (c d)"),
                    gt[:].rearrange("p c d -> p (c d)"),
                    hsel_ps[:], op=ALU.mult)
                nc.vector.tensor_reduce(
                    scorebuf[:, b, :], prod[:], axis=mybir.AxisListType.X,
                    op=ALU.add)

            nc.sync.dma_start(
                out_ext[:],
                scorebuf[:].rearrange("p b c -> p (b c)"),
            )

            pb.__exit__(None, None, None)

    nc.compile()
    return nc


_CACHE = {}
_RUN_KWARGS = {}
LAST_RESULTS = None


def _pack_idx(vals):
    """[nblk, 1024] int16 -> dma_gather SBUF layout [nblk, 128, 64]:
    index k lives at partition k%16, column k//16, replicated into all
    eight 16-partition groups."""
    nblk = vals.shape[0]
    w = vals.reshape(nblk, BLK // 16, 16).transpose(0, 2, 1)   # [nblk,16,S]
    return np.tile(w, (1, 8, 1)).astype(np.int16)


def kernel(**inputs) -> np.ndarray:
    z = np.asarray(inputs["z"], np.float32)
    pot_arcs = np.asarray(inputs["pot_arcs"])
    lin1_w = np.asarray(inputs["lin1_w"], np.float32)
    lin1_b = np.asarray(inputs["lin1_b"], np.float32)
    lin2_w = np.asarray(inputs["lin2_w"], np.float32)
    lin2_b = np.asarray(inputs["lin2_b"], np.float32)
    bil_w = np.asarray(inputs["bil_w"], np.float32)
    bil_b = np.asarray(inputs["bil_b"], np.float32)
    norm_w = np.asarray(inputs["norm_w"], np.float32)
    norm_b = np.asarray(inputs["norm_b"], np.float32)

    if not np.allclose(norm_b, 0.0):
        # general norm_b adds per-node scalar terms; not exercised by this
        # problem's inputs.  Exact numpy fallback keeps kernel() total.
        return _numpy_reference(z, pot_arcs, lin1_w, lin1_b, lin2_w, lin2_b,
                                bil_w, bil_b, norm_w, norm_b)

    w1eff = norm_w[:, None] * lin1_w.T
    b1eff = norm_b @ lin1_w.T + lin1_b
    w2eff = norm_w[:, None] * lin2_w.T
    b2eff = norm_b @ lin2_w.T + lin2_b
    wbil = bil_w[0] * norm_w[None, :]
    wcat = np.concatenate([w1eff, w2eff], axis=1).astype(_np_mdt)
    bcat = np.concatenate([b1eff, b2eff])[None, :].astype(_np_mdt)
    wbil = wbil.astype(_np_mdt)

    zpad = np.zeros((NPAD, D), np.float32)
    zpad[:N] = z
    # row remap: node n -> table row owner*SHARD + (n%SHARD)%P*SHARD_T + (n%SHARD)//P
    noff = np.arange(NPAD, dtype=np.int32) % SHARD
    rowmap = (np.arange(NPAD, dtype=np.int32) // SHARD) * SHARD + \
        (noff % P) * SHARD_T + noff // P

    a0 = pot_arcs[:, 0].astype(np.int32)
    a1 = pot_arcs[:, 1].astype(np.int32)

    # --- assign edges to tail-owner cores; tile by (head-half, tail-chunk) -
    core_of_edge = a1 // SHARD
    per_core = []
    for c in range(NCORES):
        eids = np.where(core_of_edge == c)[0]
        a0c = rowmap[a0[eids]]
        l1 = a1[eids] - c * SHARD
        bucket = (a0c >= HALF).astype(np.int64)
        chunk = l1 // P
        # sort by (bucket, chunk) and cut chunk-pure 128-edge tiles
        order = np.lexsort((chunk, bucket))
        per_core.append((eids, a0c, l1, bucket, chunk, order))

    # tiles per (bucket, chunk) must be uniform across cores (same program)
    ntile_bc = np.zeros((2, SHARD_T), np.int64)
    for c in range(NCORES):
        _, _, _, bucket, chunk, _ = per_core[c]
        for bkt in range(2):
            cnt = np.bincount(chunk[bucket == bkt], minlength=SHARD_T)
            ntile_bc[bkt] = np.maximum(ntile_bc[bkt], -(-cnt // P))
    # tile list: (bucket, chunk) repeated; pad each bucket to multiple of 8
    tiles = []
    for bkt in range(2):
        start = len(tiles)
        for ch in range(SHARD_T):
            tiles += [(bkt, ch)] * int(ntile_bc[bkt, ch])
        while (len(tiles) - start) % CB:
            tiles.append((bkt, 0))
    ntiles = len(tiles)
    nblk = ntiles // CB
    ecpad = ntiles * P

    block_specs = []
    for b in range(nblk):
        bts = tiles[b * CB:(b + 1) * CB]
        assert len({t[0] for t in bts}) == 1
        block_specs.append((bts[0][0], tuple(t[1] for t in bts)))

    # slot ranges per (bucket, chunk)
    tile_start = {}
    pos = 0
    for t in tiles:
        tile_start.setdefault(t, []).append(pos)
        pos += P

    in_maps = []
    gathers = []
    iota_col = np.arange(P, dtype=np.float32).reshape(P, 1)
    nlc = -(-nblk // LIDX_CHUNK)
    for c in range(NCORES):
        eids, a0c, l1, bucket, chunk, order = per_core[c]
        i0 = np.zeros(ecpad, np.int32)
        lidx = np.zeros(ecpad, np.int32)
        gid = np.empty(len(eids), np.int64)
        for bkt in range(2):
            for ch in range(SHARD_T):
                sel = order[(bucket[order] == bkt) & (chunk[order] == ch)]
                starts = tile_start[(bkt, ch)]
                for ti in range(len(starts)):
                    seg = sel[ti * P:(ti + 1) * P]
                    dst = starts[ti] + np.arange(len(seg))
                    i0[dst] = a0c[seg] - (HALF if bkt else 0)
                    lidx[dst] = l1[seg] - ch * P
                    gid[seg] = dst
        gathers.append((eids, gid))
        lidxcat = np.zeros((nlc * LIDX_CHUNK * BLK,), np.float32)
        lidxcat[:ecpad] = lidx.astype(np.float32)
        in_maps.append({
            "z": zpad[c * SHARD:(c + 1) * SHARD].reshape(
                SHARD_T, P, D).transpose(1, 0, 2).reshape(P, SHARD_T * D).copy(),
            "wcat": wcat,
            "bcat": bcat,
            "wbil": wbil,
            "iotac": iota_col,
            "idx0": _pack_idx(i0.astype(np.int16).reshape(nblk, BLK)).transpose(1, 0, 2).copy(),
            "lidx": lidxcat.reshape(nlc, LIDX_CHUNK * BLK).astype(_np_mdt),
        })

    key = tuple(block_specs)
    if key not in _CACHE:
        _CACHE[key] = _build(block_specs)
    nc = _CACHE[key]

    res = run_bass_kernel_spmd(nc, in_maps, list(range(NCORES)), **_RUN_KWARGS)
    global LAST_RESULTS
    LAST_RESULTS = res

    scores = np.empty(E, np.float32)
    for c in range(NCORES):
        out_c = np.asarray(res.results[c]["out"], np.float32).T.ravel()
        eids, gid = gathers[c]
        scores[eids] = out_c[gid]
    return scores + float(bil_b[0])


def _numpy_reference(z, pot_arcs, lin1_w, lin1_b, lin2_w, lin2_b,
                     bil_w, bil_b, norm_w, norm_b):
    def ln(x):
        mu = x.mean(-1, keepdims=True)
        var = x.var(-1, keepdims=True)
        return (x - mu) / np.sqrt(var + 1e-5) * norm_w + norm_b

    zn = ln(z)
    h1 = ln(np.maximum(zn @ lin1_w.T + lin1_b, 0.0))
    h2 = ln(np.maximum(zn @ lin2_w.T + lin2_b, 0.0))
    g = h1 @ bil_w[0]
    a0 = pot_arcs[:, 0].astype(np.int64)
    a1 = pot_arcs[:, 1].astype(np.int64)
    return np.einsum("ed,ed->e", g[a0], h2[a1]) + bil_b[0]
